# revision 53
# baseline (speedup 1.0000x reference)
"""Self-contained Trainium2 Bass kernel for nn_Classifier_79929341379065.

kernel(**inputs) takes FULL unsharded inputs (as produced by
reference.setup_inputs()) and returns the FULL [B, 1] float32 output.
Internally: pure data parallel over 8 NeuronCores (batch dim of x),
weights replicated.

Hardcoded shapes: B=8192, L=16, H=8, DK=DV=32, DM=256, BN=128, V=50000.
Per core: 1024 batches = 16384 tokens = 128 subtiles of 128 tokens
(each subtile = 8 attention groups of L=16).

Two-phase blocked pipeline (32 subtiles per block) so each phase gets
all 8 PSUM banks and a short dependency chain:
  phase 1 (attention): gather [n|em] rows from merged table [V,256]
    (col 0:128 = LN-normalized node_emb bf16, 128:256 = raw row with
    row 0 zeroed); PE-transpose to [nT|emT] staging; compact k|q
    projection in one PSUM bank; head-masked q4 built on DVE via
    broadcast*mask; S^T = one K=128 N=512 matmul per 4 heads with the
    additive -1e9 group/diag mask PRE-loaded into PSUM by a
    (mask, tiled-identity) matmul (start=True) so exp(PSUM) is already
    masked; PV with ones-augmented V gives [ctx~|den]; ctx staged bf16.
  phase 2 (MLPs+head): ctx/din transposes, fc1 + residual kept in PSUM
    accumulation, tanh MLPs, static path from emT; LN via bn_stats +
    Quake bit-trick rsqrt on DVE (no scalar act-table switches ever:
    scalar only runs Copy/Identity/Exp/Tanh from one table);
    logit = sum((dyn-sta)^2*wcls) with the elementwise chain on GpSimd;
    sigmoid(z) = 0.5*(1+tanh(z/2)) with the 0.5 folded into host-side
    npm/2; one [8,8] per-group aggregation matmul; single final divide.
"""

import os
import sys
import types

import numpy as np

# ---------------------------------------------------------------- constants
B, L = 8192, 16
H, DK, DV = 8, 32, 32
DM, BN, V = 256, 128, 50000
NCORES = 8
P = 128
BC = B // NCORES                  # batches per core (1024)
TOKC = BC * L                     # tokens per core (16384)
NSUB_FULL = TOKC // P             # subtiles per core (128)
GRP = P // L                      # groups per subtile (8)
ST = 4                            # subtiles per supertile
SCL = 1.0 / np.sqrt(float(DK))
EPS = 1e-5
MNEG = -1e9                       # additive mask value (exp -> 0)


def _install_ntff_hook():
    """Register the axon NTFF profiling hook if the image's antenv lacks it,
    so run_bass_kernel_spmd(trace=True) works in this container."""
    try:
        import antenv.axon_hooks  # noqa: F401
        return
    except ImportError:
        pass
    try:
        from trn_agent_boot.trn_boot import _ntff_profile_via_ctypes
        hook = _ntff_profile_via_ctypes("/opt/axon/libaxon_pjrt.so")
    except Exception:
        hook = None
    m = types.ModuleType("antenv.axon_hooks")
    m.get_axon_ntff_profile_hook = lambda: hook
    m.set_axon_ntff_profile_hook = lambda h: None
    sys.modules["antenv.axon_hooks"] = m


def _bf16(a):
    import ml_dtypes
    return np.ascontiguousarray(a.astype(ml_dtypes.bfloat16))


def _triv(g, b):
    return bool(np.allclose(g, 1.0, atol=1e-12) and np.allclose(b, 0.0, atol=1e-12))


# ------------------------------------------------------------- host weights
def _prep_consts(w):
    """Fold LN affines into projection weights; build device const arrays."""
    c = {}
    f32 = np.float32

    wq_eff = (np.asarray(w["Wq"], f32) * np.asarray(w["ln1_g"], f32)[None, :]) * SCL
    wk_eff = np.asarray(w["Wk"], f32) * np.asarray(w["ln2_g"], f32)[None, :]
    wv_eff = np.asarray(w["Wv"], f32) * np.asarray(w["ln3_g"], f32)[None, :]
    cq = (np.asarray(w["ln1_b"], f32) @ np.asarray(w["Wq"], f32).T) * SCL
    ck = np.asarray(w["ln2_b"], f32) @ np.asarray(w["Wk"], f32).T
    cv = np.asarray(w["ln3_b"], f32) @ np.asarray(w["Wv"], f32).T

    # merged compact k|q weights: [BN, 512] = [wk.T blk0|blk1 | wq.T blk0|blk1]
    c["wkq"] = _bf16(np.concatenate([wk_eff.T, wq_eff.T], axis=1))  # [BN, 512]

    c["wv"] = _bf16(wv_eff.T)                                # [BN, 256]

    c["wfc1"] = _bf16(np.asarray(w["Wfc1"], f32).T)          # [HDV, DM] rhs
    c["p1w1"] = _bf16(np.asarray(w["p1_w1"], f32).T)         # [DM, DM] lhsT
    c["p1w2"] = _bf16(np.asarray(w["p1_w2"], f32).T)         # [DM, DM] rhs
    c["p2w1"] = _bf16(np.asarray(w["p2_w1"], f32).T)         # [BN, DM] lhsT
    c["p2w2"] = _bf16(np.asarray(w["p2_w2"], f32).T)         # [DM, DM] rhs

    # merged gather table: [V, 256] = [LN-normalized | raw with row0 zeroed]
    tab = np.asarray(w["node_emb"], f32)
    m = tab.mean(axis=1, keepdims=True)
    v = ((tab - m) ** 2).mean(axis=1, keepdims=True)
    tabn = (tab - m) / np.sqrt(v + EPS)
    tabe = tab.copy()
    tabe[0, :] = 0.0
    c["tabs"] = _bf16(np.concatenate([tabn, tabe], axis=1))  # [V, 256]

    # additive mask: -1e9 where cross-group or diagonal, else 0 (symmetric)
    mb = np.full((P, P), MNEG, f32)
    for g in range(GRP):
        mb[g * L:(g + 1) * L, g * L:(g + 1) * L] = 0.0
    mb[np.eye(P, dtype=bool)] = MNEG
    c["mbneg"] = _bf16(mb)                                   # [128,128]
    c["id4"] = _bf16(np.tile(np.eye(P, dtype=f32), (1, 4)))  # [128,512]
    # 0/1 head mask for building q4 from compact q: hm[f, hh*128+t]=(f//32==hh)
    hm = np.zeros((P, 512), f32)
    for hh in range(4):
        hm[hh * 32:(hh + 1) * 32, hh * P:(hh + 1) * P] = 1.0
    c["hm4"] = _bf16(hm)                                     # [128,512]

    gind = np.zeros((P, GRP), f32)
    for g in range(GRP):
        gind[g * L:(g + 1) * L, g] = 1.0
    c["gind"] = gind

    wcls_row = np.asarray(w["Wcls"], f32).reshape(1, DM)
    c["wcls4"] = np.ascontiguousarray(
        np.broadcast_to(np.tile(wcls_row, (1, ST)), (P, ST * DM)))  # [128,1024]
    c["ident"] = _bf16(np.eye(P, dtype=f32))

    flags = {
        "qkb": not (np.allclose(cq, 0.0) and np.allclose(ck, 0.0)),
        "cv": not np.allclose(cv, 0.0),
        "p1b1": not np.allclose(w["p1_b1"], 0.0),
        "p2b1": not np.allclose(w["p2_b1"], 0.0),
        "p1b2": not np.allclose(w["p1_b2"], 0.0),
        "p2b2": not np.allclose(w["p2_b2"], 0.0),
        "p1aff": not _triv(w["p1_lng"], w["p1_lnb"]),
        "c1aff": not _triv(w["lnc1_g"], w["lnc1_b"]),
        "c2aff": not _triv(w["lnc2_g"], w["lnc2_b"]),
    }
    flags["lnc1"] = flags["p1aff"]

    if flags["qkb"]:
        # per-128-block biases for the kq copy: cols [k0,k1,q0,q1]
        kqb = np.zeros((P, 4), f32)
        kqb[:, 0], kqb[:, 1] = ck[0:128], ck[128:256]
        kqb[:, 2], kqb[:, 3] = cq[0:128], cq[128:256]
        c["kqb"] = kqb
    if flags["cv"]:
        cvb = np.zeros((P, H * 33), f32)
        for h in range(H):
            cvb[:, h * 33:h * 33 + 32] = np.broadcast_to(
                cv[h * 32:(h + 1) * 32][None, :], (P, 32))
        c["cvb"] = cvb
    if flags["p1b1"]:
        c["p1b1"] = np.stack([np.asarray(w["p1_b1"], f32)[0:128],
                              np.asarray(w["p1_b1"], f32)[128:256]], 1)
    if flags["p2b1"]:
        c["p2b1"] = np.stack([np.asarray(w["p2_b1"], f32)[0:128],
                              np.asarray(w["p2_b1"], f32)[128:256]], 1)
    if flags["p1b2"]:
        c["p1b2b"] = np.broadcast_to(
            np.asarray(w["p1_b2"], f32)[None, :], (P, DM)).copy()
    if flags["p2b2"]:
        c["p2b2b"] = np.broadcast_to(
            np.asarray(w["p2_b2"], f32)[None, :], (P, DM)).copy()
    for nm, gk, bk in (("p1", "p1_lng", "p1_lnb"), ("c1", "lnc1_g", "lnc1_b"),
                       ("c2", "lnc2_g", "lnc2_b")):
        if flags[nm + "aff"]:
            c[nm + "gb"] = np.broadcast_to(
                np.asarray(w[gk], f32)[None, :], (P, DM)).copy()
            c[nm + "bb"] = np.broadcast_to(
                np.asarray(w[bk], f32)[None, :], (P, DM)).copy()

    c["_bcls"] = float(np.asarray(w["bcls"]).reshape(-1)[0])
    c["_flags"] = flags
    return c


# ------------------------------------------------------------ device program
def build_nc(flags, bcls, n_sub, stage=8):
    import contextlib

    import concourse.bacc as bacc
    import concourse.tile as tile
    import concourse.mybir as mybir
    from concourse import bass

    dt = mybir.dt
    AF = mybir.ActivationFunctionType
    OP = mybir.AluOpType
    IOA = bass.IndirectOffsetOnAxis
    BLK = 32                       # subtiles per phase block
    assert n_sub % ST == 0
    blk = min(BLK, n_sub)
    assert n_sub % blk == 0 and blk % ST == 0

    nc = bacc.Bacc()

    # ---- dram tensors
    idxc = nc.dram_tensor("idxc", [P, n_sub], dt.int32, kind="ExternalInput")
    npmc = nc.dram_tensor("npmc", [P, n_sub], dt.float32, kind="ExternalInput")
    tabs_d = nc.dram_tensor("tabs", [V, 2 * BN], dt.bfloat16, kind="ExternalInput")
    wkq_d = nc.dram_tensor("wkq", [BN, 512], dt.bfloat16, kind="ExternalInput")
    wv_d = nc.dram_tensor("wv", [BN, 256], dt.bfloat16, kind="ExternalInput")
    wfc1_d = nc.dram_tensor("wfc1", [DM, DM], dt.bfloat16, kind="ExternalInput")
    p1w1_d = nc.dram_tensor("p1w1", [DM, DM], dt.bfloat16, kind="ExternalInput")
    p1w2_d = nc.dram_tensor("p1w2", [DM, DM], dt.bfloat16, kind="ExternalInput")
    p2w1_d = nc.dram_tensor("p2w1", [BN, DM], dt.bfloat16, kind="ExternalInput")
    p2w2_d = nc.dram_tensor("p2w2", [DM, DM], dt.bfloat16, kind="ExternalInput")
    mb_d = nc.dram_tensor("mbneg", [P, P], dt.bfloat16, kind="ExternalInput")
    id4_d = nc.dram_tensor("id4", [P, 512], dt.bfloat16, kind="ExternalInput")
    hm4_d = nc.dram_tensor("hm4", [P, 512], dt.bfloat16, kind="ExternalInput")
    gind_d = nc.dram_tensor("gind", [P, GRP], dt.float32, kind="ExternalInput")
    wcls_d = nc.dram_tensor("wcls4", [P, ST * DM], dt.float32, kind="ExternalInput")
    ident_d = nc.dram_tensor("ident", [P, P], dt.bfloat16, kind="ExternalInput")
    opt_d = {}
    for nm, shp, cond in [
        ("kqb", [P, 4], flags["qkb"]),
        ("cvb", [P, 264], flags["cv"]),
        ("p1b1", [P, 2], flags["p1b1"]), ("p2b1", [P, 2], flags["p2b1"]),
        ("p1b2b", [P, DM], flags["p1b2"]), ("p2b2b", [P, DM], flags["p2b2"]),
        ("p1gb", [P, DM], flags["p1aff"]), ("p1bb", [P, DM], flags["p1aff"]),
        ("c1gb", [P, DM], flags["c1aff"]), ("c1bb", [P, DM], flags["c1aff"]),
        ("c2gb", [P, DM], flags["c2aff"]), ("c2bb", [P, DM], flags["c2aff"]),
    ]:
        if cond:
            opt_d[nm] = nc.dram_tensor(nm, shp, dt.float32, kind="ExternalInput")
    outp = nc.dram_tensor("outp", [GRP, n_sub], dt.float32, kind="ExternalOutput")

    with tile.TileContext(nc) as tc:
        with contextlib.ExitStack() as ctx:
            singles = ctx.enter_context(tc.tile_pool(name="singles", bufs=1))
            io = ctx.enter_context(tc.tile_pool(name="io", bufs=40))
            work = ctx.enter_context(tc.tile_pool(name="work", bufs=6))

            def load(d, shape, dtp):
                t = singles.tile(shape, dtp, name=d.name + "_sb")
                nc.sync.dma_start(t[:], d[:, :])
                return t

            idx_sb = load(idxc, [P, n_sub], dt.int32)
            npm_sb = load(npmc, [P, n_sub], dt.float32)
            wkq = load(wkq_d, [BN, 512], dt.bfloat16)
            wv = load(wv_d, [BN, 256], dt.bfloat16)
            mb_sb = load(mb_d, [P, P], dt.bfloat16)
            id4_sb = load(id4_d, [P, 512], dt.bfloat16)
            hm4_sb = load(hm4_d, [P, 512], dt.bfloat16)
            gind_sb = load(gind_d, [P, GRP], dt.float32)
            wcls_sb = load(wcls_d, [P, ST * DM], dt.float32)
            ident = load(ident_d, [P, P], dt.bfloat16)
            wfc1, p1w1, p1w2, p2w2 = ([None, None] for _ in range(4))
            for k in range(2):
                for nm, arr, d in (("wfc1", wfc1, wfc1_d), ("p1w1", p1w1, p1w1_d),
                                   ("p1w2", p1w2, p1w2_d), ("p2w2", p2w2, p2w2_d)):
                    arr[k] = singles.tile([P, DM], dt.bfloat16, name=f"{nm}_{k}")
                    nc.sync.dma_start(arr[k][:], d[k * P:(k + 1) * P, :])
            p2w1 = load(p2w1_d, [BN, DM], dt.bfloat16)
            osb = {nm: load(d, d.shape, dt.float32) for nm, d in opt_d.items()}

            res = singles.tile([GRP, 2 * n_sub], dt.float32, name="res")
            cmagic = singles.tile([P, 2 * ST], dt.int32, name="cmagic")
            nc.vector.memset(cmagic[:], 0x5F3759DF)

            # block staging: [nT | emT] and ctx, per subtile 256 bf16 cols
            nem_stage = singles.tile([P, blk * 256], dt.bfloat16, name="nem_stage")
            ctx_stage = singles.tile([P, blk * 256], dt.bfloat16, name="ctx_stage")

            def rsqrt_dve(out_ap, var_ap, n):
                """out = 1/sqrt(var+eps) on DVE only (no scalar act tables):
                Quake bit-trick seed + 2 Newton iterations."""
                ve = work.tile([P, 2 * ST], dt.float32, tag="rsq_ve", name="ve")
                vea = ve[:, 0:n]
                nc.vector.tensor_scalar_add(vea, var_ap, EPS)
                shi = work.tile([P, 2 * ST], dt.int32, tag="rsq_sh", name="shi")
                nc.vector.tensor_scalar(
                    out=shi[:, 0:n], in0=vea.bitcast(dt.int32), scalar1=1,
                    scalar2=None, op0=OP.logical_shift_right)
                z = work.tile([P, 2 * ST], dt.float32, tag="rsq_z", name="z")
                nc.vector.tensor_tensor(out=z[:, 0:n].bitcast(dt.int32),
                                        in0=cmagic[:, 0:n], in1=shi[:, 0:n],
                                        op=OP.subtract)
                t = work.tile([P, 2 * ST], dt.float32, tag="rsq_t", name="t")
                for _ in range(1):
                    nc.vector.tensor_tensor(out=t[:, 0:n], in0=vea,
                                            in1=z[:, 0:n], op=OP.mult)
                    nc.vector.tensor_tensor(out=t[:, 0:n], in0=t[:, 0:n],
                                            in1=z[:, 0:n], op=OP.mult)
                    nc.vector.tensor_scalar(out=t[:, 0:n], in0=t[:, 0:n],
                                            scalar1=-0.5, scalar2=1.5,
                                            op0=OP.mult, op1=OP.add)
                    nc.vector.tensor_tensor(out=z[:, 0:n], in0=z[:, 0:n],
                                            in1=t[:, 0:n], op=OP.mult)
                nc.vector.tensor_copy(out_ap, z[:, 0:n])

            ne_pend = []

            def emit_gathers(bb):
                for i2 in range(blk):
                    t2 = bb * blk + i2
                    net = io.tile([P, 256], dt.bfloat16, tag="ne", name="ne")
                    nc.gpsimd.indirect_dma_start(
                        out=net[:], out_offset=None, in_=tabs_d[:, :],
                        in_offset=IOA(ap=idx_sb[:, t2:t2 + 1], axis=0))
                    ne_pend.append(net)

            emit_gathers(0)
            for b in range(n_sub // blk):
                # ================= phase 1: attention -> ctx/nem staging
                with contextlib.ExitStack() as c1:
                    # mix bank: [0:128 fp32-cols = ne-transpose (bf16 view),
                    #            128:384 = v] ; kq and ca share one tag
                    mixp = c1.enter_context(
                        tc.tile_pool(name="mixp", bufs=2, space="PSUM"))
                    kqca = c1.enter_context(
                        tc.tile_pool(name="kqca", bufs=3, space="PSUM"))
                    sp = c1.enter_context(
                        tc.tile_pool(name="sp", bufs=3, space="PSUM"))
                    for i in range(blk):
                        t = b * blk + i
                        stg = slice(i * 256, (i + 1) * 256)

                        ne = ne_pend.pop(0)

                        mix = mixp.tile([P, 384], dt.float32, tag="mix",
                                        name="mix")
                        ne_ps = mix[:, 0:128].bitcast(dt.bfloat16)
                        nc.tensor.transpose(ne_ps[:, 0:P], ne[:, 0:P], ident[:])
                        nc.tensor.transpose(ne_ps[:, P:256], ne[:, P:256],
                                            ident[:])
                        nc.vector.tensor_copy(nem_stage[:, stg], ne_ps[:])
                        nT = nem_stage[:, i * 256:i * 256 + P]

                        kq_ps = kqca.tile([P, 512], dt.float32, tag="kq",
                                          name="kq_ps")
                        for j in range(4):
                            nc.tensor.matmul(kq_ps[:, j * P:(j + 1) * P],
                                             lhsT=wkq[:, j * P:(j + 1) * P],
                                             rhs=nT)
                        kqT = work.tile([P, 512], dt.bfloat16, tag="kqT",
                                        name="kqT")
                        if flags["qkb"]:
                            for j in range(4):
                                nc.scalar.activation(
                                    kqT[:, j * P:(j + 1) * P],
                                    kq_ps[:, j * P:(j + 1) * P],
                                    AF.Identity, bias=osb["kqb"][:, j:j + 1])
                        else:
                            nc.scalar.activation(kqT[:], kq_ps[:], AF.Copy)

                        v_ps = mix[:, 128:384]
                        nc.tensor.matmul(v_ps, lhsT=nT, rhs=wv[:])
                        v_aug = work.tile([P, 264], dt.bfloat16, tag="v_aug",
                                          name="v_aug")
                        va3 = v_aug[:].rearrange("p (h c) -> p h c", c=33)
                        nc.vector.tensor_copy(
                            va3[:, :, 0:32],
                            v_ps.rearrange("p (h c) -> p h c", c=32))
                        if flags["cv"]:
                            nc.vector.tensor_add(v_aug[:], v_aug[:],
                                                 osb["cvb"][:])
                        nc.vector.memset(va3[:, :, 32:33], 1.0)

                        pt = work.tile([P, 1024], dt.bfloat16, tag="pt",
                                       name="pt")
                        for b2 in range(2):
                            q4m = work.tile([P, 512], dt.bfloat16, tag="q4m",
                                            name="q4m")
                            qv = kqT[:, 256 + b2 * P:256 + (b2 + 1) * P]
                            nc.vector.tensor_tensor(
                                out=q4m[:].rearrange("p (j t) -> p j t", j=4),
                                in0=qv.unsqueeze(1).to_broadcast([P, 4, P]),
                                in1=hm4_sb[:].rearrange("p (j t) -> p j t", j=4),
                                op=OP.mult)
                            s_ps = sp.tile([P, 512], dt.float32, tag="s",
                                           name="s_ps")
                            nc.tensor.matmul(s_ps[:], lhsT=mb_sb[:],
                                             rhs=id4_sb[:], start=True,
                                             stop=False)
                            nc.tensor.matmul(s_ps[:],
                                             lhsT=kqT[:, b2 * P:(b2 + 1) * P],
                                             rhs=q4m[:], start=False, stop=True)
                            nc.scalar.activation(pt[:, b2 * 512:(b2 + 1) * 512],
                                                 s_ps[:], AF.Exp)

                        ca_ps = kqca.tile([P, 512], dt.float32, tag="kq",
                                          name="ca_ps")
                        for h in range(H):
                            nc.tensor.matmul(
                                ca_ps[:, h * 33:(h + 1) * 33],
                                lhsT=pt[:, h * P:(h + 1) * P],
                                rhs=v_aug[:, h * 33:(h + 1) * 33])
                        ca3 = ca_ps[:, 0:264].rearrange("p (h c) -> p h c", c=33)
                        rec = work.tile([P, H], dt.float32, tag="rec",
                                        name="rec")
                        rec3 = rec[:].rearrange("p (h o) -> p h o", o=1)
                        nc.vector.reciprocal(rec3[:], ca3[:, :, 32:33])
                        cs3 = ctx_stage[:, stg].rearrange("p (h c) -> p h c",
                                                          c=32)
                        nc.vector.tensor_tensor(
                            out=cs3[:], in0=ca3[:, :, 0:32],
                            in1=rec3.to_broadcast([P, H, 32]), op=OP.mult)

                # ================= phase 2: MLPs + classifier head
                with contextlib.ExitStack() as c2:
                    # prefetch next block's gather rows while the MLP phase
                    # runs (gpsimd is mostly idle here)
                    if b + 1 < n_sub // blk:
                        emit_gathers(b + 1)
                    # bank packing: mix2 = [ct-transpose (bf16 view) | dinT],
                    # dust = [du | st], hh12 = [h1 | h2] -- one alloc per
                    # subtile per tag, so every tag has >=2 subtiles lookahead
                    mix2 = c2.enter_context(
                        tc.tile_pool(name="mix2", bufs=3, space="PSUM"))
                    dust = c2.enter_context(
                        tc.tile_pool(name="dust", bufs=2, space="PSUM"))
                    hh12 = c2.enter_context(
                        tc.tile_pool(name="hh12", bufs=3, space="PSUM"))
                    pend = []

                    def emit_tail(gss, mv_st, u2_st, st2_st):
                            # ======== supertile tail ========
                            mvv = mv_st[:].rearrange("p (j k) -> p j k", k=2)
                            rstd_st = work.tile([P, 2 * ST], dt.float32,
                                                tag="rstds", name="rstd_st")
                            rsqrt_dve(rstd_st[:], mvv[:, :, 1:2].squeeze(2),
                                      2 * ST)
                            # negmrs[:, 2s] = -mu*ru, [:, 2s+1] = -ms*rs, so the
                            # dyn LN-apply can run on Scalar as Identity(u*r + b)
                            nrstd = work.tile([P, 2 * ST], dt.float32, tag="nrstd",
                                              name="nrstd")
                            nc.vector.tensor_scalar_mul(nrstd[:], rstd_st[:], -1.0)
                            negmrs = work.tile([P, 2 * ST], dt.float32,
                                               tag="negmrs", name="negmrs")
                            nc.vector.tensor_tensor(
                                out=negmrs[:].rearrange("p (j o) -> p j o", o=1),
                                in0=mvv[:, :, 0:1], in1=nrstd[:].rearrange(
                                    "p (j o) -> p j o", o=1), op=OP.mult)

                            dyn_st = work.tile([P, ST * DM], dt.float32, tag="dyn",
                                               name="dyn_st", bufs=2)
                            sta_st = work.tile([P, ST * DM], dt.float32, tag="sta",
                                               name="sta_st", bufs=2)
                            for s in range(ST):
                                dyn_sl = dyn_st[:, s * DM:(s + 1) * DM]
                                nc.scalar.activation(
                                    dyn_sl, u2_st[:, s * DM:(s + 1) * DM],
                                    AF.Identity,
                                    bias=negmrs[:, 2 * s:2 * s + 1],
                                    scale=rstd_st[:, 2 * s:2 * s + 1])
                                if flags["p1aff"]:
                                    nc.vector.tensor_mul(dyn_sl, dyn_sl,
                                                             osb["p1gb"][:])
                                    nc.vector.tensor_add(dyn_sl, dyn_sl,
                                                             osb["p1bb"][:])
                                    nc.vector.tensor_scalar(
                                        out=dyn_sl, in0=dyn_sl,
                                        scalar1=npm_sb[:, gss * ST + s:
                                                           gss * ST + s + 1],
                                        scalar2=2.0, op0=OP.mult, op1=OP.mult)
                                if flags["lnc1"]:
                                    st6c = work.tile([P, 6], dt.float32,
                                                         tag="st6c", name="st6c")
                                    nc.vector.bn_stats(st6c[:], dyn_sl)
                                    mvc = work.tile([P, 2], dt.float32, tag="mvc",
                                                        name="mvc")
                                    nc.vector.bn_aggr(mvc[:], st6c[:])
                                    rstdc = work.tile([P, 1], dt.float32,
                                                          tag="rstdc", name="rstdc")
                                    rsqrt_dve(rstdc[:], mvc[:, 1:2], 1)
                                    nc.vector.tensor_scalar(
                                        out=dyn_sl, in0=dyn_sl,
                                        scalar1=mvc[:, 0:1], scalar2=rstdc[:, 0:1],
                                        op0=OP.subtract, op1=OP.mult)
                                if flags["c1aff"]:
                                    nc.vector.tensor_mul(dyn_sl, dyn_sl,
                                                             osb["c1gb"][:])
                                    nc.vector.tensor_add(dyn_sl, dyn_sl,
                                                             osb["c1bb"][:])

                                sta_sl = sta_st[:, s * DM:(s + 1) * DM]
                                nc.scalar.activation(
                                    sta_sl, st2_st[:, s * DM:(s + 1) * DM],
                                    AF.Identity,
                                    bias=negmrs[:, 2 * s + 1:2 * s + 2],
                                    scale=rstd_st[:, 2 * s + 1:2 * s + 2])
                                if flags["c2aff"]:
                                    nc.vector.tensor_mul(sta_sl, sta_sl,
                                                             osb["c2gb"][:])
                                    nc.vector.tensor_add(sta_sl, sta_sl,
                                                             osb["c2bb"][:])

                            dd_st = work.tile([P, ST * DM], dt.float32, tag="dd",
                                              name="dd_st", bufs=2)
                            nc.gpsimd.tensor_tensor(out=dd_st[:], in0=dyn_st[:],
                                                        in1=sta_st[:], op=OP.subtract)
                            dw_st = work.tile([P, ST * DM], dt.float32, tag="dw",
                                              name="dw_st", bufs=2)
                            nc.gpsimd.tensor_tensor(out=dw_st[:], in0=dd_st[:],
                                                        in1=wcls_sb[:], op=OP.mult)
                            wsq_st = work.tile([P, ST * DM], dt.float32, tag="wsq",
                                               name="wsq_st", bufs=2)
                            nc.gpsimd.tensor_tensor(out=wsq_st[:], in0=dd_st[:],
                                                        in1=dw_st[:], op=OP.mult)
                            logit_st = work.tile([P, ST], dt.float32, tag="lg",
                                                 name="logit_st")
                            nc.vector.tensor_reduce(
                                logit_st[:],
                                wsq_st[:].rearrange("p (s d) -> p s d", d=DM),
                                axis=mybir.AxisListType.X, op=OP.add)

                            # sigmoid(z) = 0.5*(1+tanh(z/2)); npm_sb holds npm/2
                            t_st = work.tile([P, ST], dt.float32, tag="tst",
                                             name="t_st")
                            nc.scalar.activation(t_st[:], logit_st[:], AF.Tanh,
                                                 bias=0.5 * bcls, scale=0.5)
                            t1_st = work.tile([P, ST], dt.float32, tag="t1st",
                                              name="t1_st")
                            nc.vector.tensor_scalar_add(t1_st[:], t_st[:], 1.0)

                            npm4 = npm_sb[:, gss * ST:(gss + 1) * ST]
                            pn_st = work.tile([P, 2 * ST], dt.float32, tag="pn",
                                              name="pn_st")
                            pnv = pn_st[:].rearrange("p (s k) -> p s k", k=2)
                            nc.vector.tensor_tensor(
                                out=pnv[:, :, 0:1],
                                in0=t1_st[:].rearrange("p (s o) -> p s o", o=1),
                                in1=npm4.rearrange("p (s o) -> p s o", o=1),
                                op=OP.mult)
                            nc.gpsimd.tensor_scalar_mul(
                                pnv[:, :, 1:2],
                                npm4.rearrange("p (s o) -> p s o", o=1), 2.0)

                            agg_ps = dust.tile([GRP, 2 * ST], dt.float32,
                                               tag="ds", name="agg_ps")
                            nc.tensor.matmul(agg_ps[:], lhsT=gind_sb[:],
                                             rhs=pn_st[:])
                            nc.scalar.activation(
                                res[0:GRP, 2 * ST * gss:2 * ST * (gss + 1)],
                                agg_ps[:], AF.Copy)

                    for ss in range(blk // ST):
                        gss = b * (blk // ST) + ss
                        mv_st = work.tile([P, 4 * ST], dt.float32, tag="mv",
                                          name="mv_st", bufs=2)
                        u2_st = work.tile([P, ST * DM], dt.float32, tag="u2",
                                          name="u2_st", bufs=2)
                        st2_st = work.tile([P, ST * DM], dt.float32, tag="st2",
                                           name="st2_st", bufs=2)
                        for s in range(ST):
                            i = ss * ST + s
                            ctxs = ctx_stage[:, i * 256:(i + 1) * 256]
                            emT = nem_stage[:, i * 256 + P:(i + 1) * 256]

                            m2 = mix2.tile([P, 384], dt.float32, tag="m2",
                                           name="m2")
                            ct_ps = m2[:, 0:128].bitcast(dt.bfloat16)
                            nc.tensor.transpose(ct_ps[:, 0:P], ctxs[:, 0:P],
                                                ident[:])
                            nc.tensor.transpose(ct_ps[:, P:256], ctxs[:, P:256],
                                                ident[:])
                            ctxT = work.tile([P, 256], dt.bfloat16, tag="ctxT",
                                             name="ctxT")
                            nc.vector.tensor_copy(ctxT[:], ct_ps[:])

                            ds = dust.tile([P, 512], dt.float32, tag="ds",
                                           name="ds")
                            du_ps = ds[:, 0:256]
                            nc.tensor.matmul(du_ps, lhsT=ctxT[:, 0:P],
                                             rhs=wfc1[0][:], start=True,
                                             stop=False)
                            nc.tensor.matmul(du_ps, lhsT=ctxT[:, P:256],
                                             rhs=wfc1[1][:], start=False,
                                             stop=True)

                            # flipped fc1: dinT computed weight-stationary,
                            # no transpose of dyn_in needed
                            dinT_ps = m2[:, 128:384]
                            for f in range(2):
                                for k in range(2):
                                    nc.tensor.matmul(
                                        dinT_ps[:, f * P:(f + 1) * P],
                                        lhsT=wfc1[k][:, f * P:(f + 1) * P],
                                        rhs=ctxT[:, k * P:(k + 1) * P],
                                        start=(k == 0), stop=(k == 1))
                            dinT = work.tile([P, 256], dt.bfloat16, tag="dinT",
                                             name="dinT")
                            nc.vector.tensor_copy(dinT[:], dinT_ps)

                            hh = hh12.tile([P, 512], dt.float32, tag="hh",
                                           name="hh")
                            h1_ps = hh[:, 0:256]
                            for f in range(2):
                                for k in range(2):
                                    nc.tensor.matmul(
                                        h1_ps[:, f * P:(f + 1) * P],
                                        lhsT=p1w1[k][:, f * P:(f + 1) * P],
                                        rhs=dinT[:, k * P:(k + 1) * P],
                                        start=(k == 0), stop=(k == 1))
                            h1T = work.tile([P, DM], dt.bfloat16, tag="h1T",
                                            name="h1T")
                            if flags["p1b1"]:
                                for f in range(2):
                                    nc.scalar.activation(
                                        h1T[:, f * P:(f + 1) * P],
                                        h1_ps[:, f * P:(f + 1) * P], AF.Tanh,
                                        bias=osb["p1b1"][:, f:f + 1])
                            else:
                                nc.scalar.activation(h1T[:], h1_ps[:], AF.Tanh)

                            nc.tensor.matmul(du_ps, lhsT=h1T[:, 0:P],
                                             rhs=p1w2[0][:], start=False,
                                             stop=False, skip_group_check=True)
                            nc.tensor.matmul(du_ps, lhsT=h1T[:, P:256],
                                             rhs=p1w2[1][:], start=False,
                                             stop=True, skip_group_check=True)
                            if flags["p1b2"]:
                                nc.vector.tensor_add(du_ps, du_ps,
                                                     osb["p1b2b"][:])
                            u_sl = u2_st[:, s * DM:(s + 1) * DM]
                            st6a = work.tile([P, 6], dt.float32, tag="st6a",
                                             name="st6a")
                            nc.vector.bn_stats(st6a[:], du_ps)
                            nc.scalar.activation(u_sl, du_ps, AF.Copy)
                            nc.vector.bn_aggr(mv_st[:, 4 * s:4 * s + 2],
                                              st6a[:])

                            h2_ps = hh[:, 256:512]
                            for f in range(2):
                                nc.tensor.matmul(h2_ps[:, f * P:(f + 1) * P],
                                                 lhsT=p2w1[:, f * P:(f + 1) * P],
                                                 rhs=emT)
                            h2T = work.tile([P, DM], dt.bfloat16, tag="h2T",
                                            name="h2T")
                            if flags["p2b1"]:
                                for f in range(2):
                                    nc.scalar.activation(
                                        h2T[:, f * P:(f + 1) * P],
                                        h2_ps[:, f * P:(f + 1) * P], AF.Tanh,
                                        bias=osb["p2b1"][:, f:f + 1])
                            else:
                                nc.scalar.activation(h2T[:], h2_ps[:], AF.Tanh)

                            st_ps = ds[:, 256:512]
                            nc.tensor.matmul(st_ps, lhsT=h2T[:, 0:P],
                                             rhs=p2w2[0][:], start=True,
                                             stop=False)
                            nc.tensor.matmul(st_ps, lhsT=h2T[:, P:256],
                                             rhs=p2w2[1][:], start=False,
                                             stop=True)
                            if flags["p2b2"]:
                                nc.vector.tensor_add(st_ps, st_ps,
                                                     osb["p2b2b"][:])
                            st_sl = st2_st[:, s * DM:(s + 1) * DM]
                            st6b = work.tile([P, 6], dt.float32, tag="st6b",
                                             name="st6b")
                            nc.vector.bn_stats(st6b[:], st_ps)
                            nc.scalar.activation(st_sl, st_ps, AF.Copy)
                            nc.vector.bn_aggr(mv_st[:, 4 * s + 2:4 * s + 4],
                                              st6b[:])

                        # defer this supertile's tail by one supertile so the
                        # serial gpsimd dd/dw/wsq chain never head-of-line
                        # blocks the vector queue at tensor_reduce
                        pend.append((gss, mv_st, u2_st, st2_st))
                        if len(pend) > 1:
                            emit_tail(*pend.pop(0))


                    while pend:
                        emit_tail(*pend.pop(0))

            # ---- final divide + store
            r3 = res[:].rearrange("p (t k) -> p t k", k=2)
            rn = work.tile([GRP, n_sub], dt.float32, tag="rn", name="rn")
            rn3 = rn[:].rearrange("p (t o) -> p t o", o=1)
            nc.vector.reciprocal(rn3[:], r3[:, :, 1:2])
            orow = work.tile([GRP, n_sub], dt.float32, tag="orow", name="orow")
            orow3 = orow[:].rearrange("p (t o) -> p t o", o=1)
            nc.vector.tensor_tensor(out=orow3[:], in0=r3[:, :, 0:1], in1=rn3[:],
                                    op=OP.mult)
            nc.sync.dma_start(outp[:, :], orow[:])

    nc.finalize()
    return nc


# ----------------------------------------------------------------- entry
_NC_CACHE = {}


def kernel(**inputs):
    _install_ntff_hook()
    from concourse.bass_utils import run_bass_kernel_spmd

    n_sub = int(os.environ.get("KBENCH_NSUB", NSUB_FULL))
    consts = _prep_consts(inputs)
    flags = consts.pop("_flags")
    bcls = consts.pop("_bcls")

    stage = int(os.environ.get("KBENCH_STAGE", "8"))
    key = (n_sub, stage, tuple(sorted(flags.items())))
    if key not in _NC_CACHE:
        _NC_CACHE[key] = build_nc(flags, bcls, n_sub, stage)
    nc = _NC_CACHE[key]

    x = np.asarray(inputs["x"]).astype(np.int32)
    in_maps = []
    for c in range(NCORES):
        xc = x[c * BC:(c + 1) * BC].reshape(-1)          # [16384]
        idxc = np.ascontiguousarray(
            xc[:n_sub * P].reshape(n_sub, P).T)          # [128, n_sub]
        m = {"idxc": idxc, "npmc": 0.5 * (idxc != 0).astype(np.float32)}
        m.update(consts)
        in_maps.append(m)

    trace = bool(int(os.environ.get("KBENCH_TRACE", "0")))
    res = run_bass_kernel_spmd(nc, in_maps, core_ids=list(range(NCORES)),
                               trace=trace)
    kernel._last_results = res

    out = np.zeros((B, 1), np.float32)
    for c in range(NCORES):
        oc = res.results[c]["outp"]                      # [8, n_sub]
        out[c * BC:c * BC + n_sub * GRP, 0] = oc.T.reshape(-1)
    return out


# revision 54
# speedup vs baseline: 1.0071x; 1.0071x over previous
"""Self-contained Trainium2 Bass kernel for nn_Classifier_79929341379065.

kernel(**inputs) takes FULL unsharded inputs (as produced by
reference.setup_inputs()) and returns the FULL [B, 1] float32 output.
Internally: pure data parallel over 8 NeuronCores (batch dim of x),
weights replicated.

Hardcoded shapes: B=8192, L=16, H=8, DK=DV=32, DM=256, BN=128, V=50000.
Per core: 1024 batches = 16384 tokens = 128 subtiles of 128 tokens
(each subtile = 8 attention groups of L=16).

Two-phase blocked pipeline (32 subtiles per block) so each phase gets
all 8 PSUM banks and a short dependency chain:
  phase 1 (attention): gather [n|em] rows from merged table [V,256]
    (col 0:128 = LN-normalized node_emb bf16, 128:256 = raw row with
    row 0 zeroed); PE-transpose to [nT|emT] staging; compact k|q
    projection in one PSUM bank; head-masked q4 built on DVE via
    broadcast*mask; S^T = one K=128 N=512 matmul per 4 heads with the
    additive -1e9 group/diag mask PRE-loaded into PSUM by a
    (mask, tiled-identity) matmul (start=True) so exp(PSUM) is already
    masked; PV with ones-augmented V gives [ctx~|den]; ctx staged bf16.
  phase 2 (MLPs+head): ctx/din transposes, fc1 + residual kept in PSUM
    accumulation, tanh MLPs, static path from emT; LN via bn_stats +
    Quake bit-trick rsqrt on DVE (no scalar act-table switches ever:
    scalar only runs Copy/Identity/Exp/Tanh from one table);
    logit = sum((dyn-sta)^2*wcls) with the elementwise chain on GpSimd;
    sigmoid(z) = 0.5*(1+tanh(z/2)) with the 0.5 folded into host-side
    npm/2; one [8,8] per-group aggregation matmul; single final divide.
"""

import os
import sys
import types

import numpy as np

# ---------------------------------------------------------------- constants
B, L = 8192, 16
H, DK, DV = 8, 32, 32
DM, BN, V = 256, 128, 50000
NCORES = 8
P = 128
BC = B // NCORES                  # batches per core (1024)
TOKC = BC * L                     # tokens per core (16384)
NSUB_FULL = TOKC // P             # subtiles per core (128)
GRP = P // L                      # groups per subtile (8)
ST = 4                            # subtiles per supertile
SCL = 1.0 / np.sqrt(float(DK))
EPS = 1e-5
MNEG = -1e9                       # additive mask value (exp -> 0)


def _install_ntff_hook():
    """Register the axon NTFF profiling hook if the image's antenv lacks it,
    so run_bass_kernel_spmd(trace=True) works in this container."""
    try:
        import antenv.axon_hooks  # noqa: F401
        return
    except ImportError:
        pass
    try:
        from trn_agent_boot.trn_boot import _ntff_profile_via_ctypes
        hook = _ntff_profile_via_ctypes("/opt/axon/libaxon_pjrt.so")
    except Exception:
        hook = None
    m = types.ModuleType("antenv.axon_hooks")
    m.get_axon_ntff_profile_hook = lambda: hook
    m.set_axon_ntff_profile_hook = lambda h: None
    sys.modules["antenv.axon_hooks"] = m


def _bf16(a):
    import ml_dtypes
    return np.ascontiguousarray(a.astype(ml_dtypes.bfloat16))


def _triv(g, b):
    return bool(np.allclose(g, 1.0, atol=1e-12) and np.allclose(b, 0.0, atol=1e-12))


# ------------------------------------------------------------- host weights
def _prep_consts(w):
    """Fold LN affines into projection weights; build device const arrays."""
    c = {}
    f32 = np.float32

    wq_eff = (np.asarray(w["Wq"], f32) * np.asarray(w["ln1_g"], f32)[None, :]) * SCL
    wk_eff = np.asarray(w["Wk"], f32) * np.asarray(w["ln2_g"], f32)[None, :]
    wv_eff = np.asarray(w["Wv"], f32) * np.asarray(w["ln3_g"], f32)[None, :]
    cq = (np.asarray(w["ln1_b"], f32) @ np.asarray(w["Wq"], f32).T) * SCL
    ck = np.asarray(w["ln2_b"], f32) @ np.asarray(w["Wk"], f32).T
    cv = np.asarray(w["ln3_b"], f32) @ np.asarray(w["Wv"], f32).T

    # merged compact k|q weights: [BN, 512] = [wk.T blk0|blk1 | wq.T blk0|blk1]
    c["wkq"] = _bf16(np.concatenate([wk_eff.T, wq_eff.T], axis=1))  # [BN, 512]

    c["wv"] = _bf16(wv_eff.T)                                # [BN, 256]

    c["wfc1"] = _bf16(np.asarray(w["Wfc1"], f32).T)          # [HDV, DM] rhs
    c["p1w1"] = _bf16(np.asarray(w["p1_w1"], f32).T)         # [DM, DM] lhsT
    c["p1w2"] = _bf16(np.asarray(w["p1_w2"], f32).T)         # [DM, DM] rhs
    c["p2w1"] = _bf16(np.asarray(w["p2_w1"], f32).T)         # [BN, DM] lhsT
    c["p2w2"] = _bf16(np.asarray(w["p2_w2"], f32).T)         # [DM, DM] rhs

    # merged gather table: [V, 256] = [LN-normalized | raw with row0 zeroed]
    tab = np.asarray(w["node_emb"], f32)
    m = tab.mean(axis=1, keepdims=True)
    v = ((tab - m) ** 2).mean(axis=1, keepdims=True)
    tabn = (tab - m) / np.sqrt(v + EPS)
    tabe = tab.copy()
    tabe[0, :] = 0.0
    c["tabs"] = _bf16(np.concatenate([tabn, tabe], axis=1))  # [V, 256]

    # additive mask: -1e9 where cross-group or diagonal, else 0 (symmetric)
    mb = np.full((P, P), MNEG, f32)
    for g in range(GRP):
        mb[g * L:(g + 1) * L, g * L:(g + 1) * L] = 0.0
    mb[np.eye(P, dtype=bool)] = MNEG
    c["mbneg"] = _bf16(mb)                                   # [128,128]
    c["id4"] = _bf16(np.tile(np.eye(P, dtype=f32), (1, 4)))  # [128,512]
    # 0/1 head mask for building q4 from compact q: hm[f, hh*128+t]=(f//32==hh)
    hm = np.zeros((P, 512), f32)
    for hh in range(4):
        hm[hh * 32:(hh + 1) * 32, hh * P:(hh + 1) * P] = 1.0
    c["hm4"] = _bf16(hm)                                     # [128,512]

    gind = np.zeros((P, GRP), f32)
    for g in range(GRP):
        gind[g * L:(g + 1) * L, g] = 1.0
    c["gind"] = gind

    wcls_row = np.asarray(w["Wcls"], f32).reshape(1, DM)
    c["wcls4"] = np.ascontiguousarray(
        np.broadcast_to(np.tile(wcls_row, (1, ST)), (P, ST * DM)))  # [128,1024]
    c["ident"] = _bf16(np.eye(P, dtype=f32))

    flags = {
        "qkb": not (np.allclose(cq, 0.0) and np.allclose(ck, 0.0)),
        "cv": not np.allclose(cv, 0.0),
        "p1b1": not np.allclose(w["p1_b1"], 0.0),
        "p2b1": not np.allclose(w["p2_b1"], 0.0),
        "p1b2": not np.allclose(w["p1_b2"], 0.0),
        "p2b2": not np.allclose(w["p2_b2"], 0.0),
        "p1aff": not _triv(w["p1_lng"], w["p1_lnb"]),
        "c1aff": not _triv(w["lnc1_g"], w["lnc1_b"]),
        "c2aff": not _triv(w["lnc2_g"], w["lnc2_b"]),
    }
    flags["lnc1"] = flags["p1aff"]

    if flags["qkb"]:
        # per-128-block biases for the kq copy: cols [k0,k1,q0,q1]
        kqb = np.zeros((P, 4), f32)
        kqb[:, 0], kqb[:, 1] = ck[0:128], ck[128:256]
        kqb[:, 2], kqb[:, 3] = cq[0:128], cq[128:256]
        c["kqb"] = kqb
    if flags["cv"]:
        cvb = np.zeros((P, H * 33), f32)
        for h in range(H):
            cvb[:, h * 33:h * 33 + 32] = np.broadcast_to(
                cv[h * 32:(h + 1) * 32][None, :], (P, 32))
        c["cvb"] = cvb
    if flags["p1b1"]:
        c["p1b1"] = np.stack([np.asarray(w["p1_b1"], f32)[0:128],
                              np.asarray(w["p1_b1"], f32)[128:256]], 1)
    if flags["p2b1"]:
        c["p2b1"] = np.stack([np.asarray(w["p2_b1"], f32)[0:128],
                              np.asarray(w["p2_b1"], f32)[128:256]], 1)
    if flags["p1b2"]:
        c["p1b2b"] = np.broadcast_to(
            np.asarray(w["p1_b2"], f32)[None, :], (P, DM)).copy()
    if flags["p2b2"]:
        c["p2b2b"] = np.broadcast_to(
            np.asarray(w["p2_b2"], f32)[None, :], (P, DM)).copy()
    for nm, gk, bk in (("p1", "p1_lng", "p1_lnb"), ("c1", "lnc1_g", "lnc1_b"),
                       ("c2", "lnc2_g", "lnc2_b")):
        if flags[nm + "aff"]:
            c[nm + "gb"] = np.broadcast_to(
                np.asarray(w[gk], f32)[None, :], (P, DM)).copy()
            c[nm + "bb"] = np.broadcast_to(
                np.asarray(w[bk], f32)[None, :], (P, DM)).copy()

    c["_bcls"] = float(np.asarray(w["bcls"]).reshape(-1)[0])
    c["_flags"] = flags
    return c


# ------------------------------------------------------------ device program
def build_nc(flags, bcls, n_sub, stage=8):
    import contextlib

    import concourse.bacc as bacc
    import concourse.tile as tile
    import concourse.mybir as mybir
    from concourse import bass

    dt = mybir.dt
    AF = mybir.ActivationFunctionType
    OP = mybir.AluOpType
    IOA = bass.IndirectOffsetOnAxis
    BLK = 32                       # subtiles per phase block
    assert n_sub % ST == 0
    blk = min(BLK, n_sub)
    assert n_sub % blk == 0 and blk % ST == 0

    nc = bacc.Bacc()

    # ---- dram tensors
    idxc = nc.dram_tensor("idxc", [P, n_sub], dt.int32, kind="ExternalInput")
    npmc = nc.dram_tensor("npmc", [P, n_sub], dt.float32, kind="ExternalInput")
    tabs_d = nc.dram_tensor("tabs", [V, 2 * BN], dt.bfloat16, kind="ExternalInput")
    wkq_d = nc.dram_tensor("wkq", [BN, 512], dt.bfloat16, kind="ExternalInput")
    wv_d = nc.dram_tensor("wv", [BN, 256], dt.bfloat16, kind="ExternalInput")
    wfc1_d = nc.dram_tensor("wfc1", [DM, DM], dt.bfloat16, kind="ExternalInput")
    p1w1_d = nc.dram_tensor("p1w1", [DM, DM], dt.bfloat16, kind="ExternalInput")
    p1w2_d = nc.dram_tensor("p1w2", [DM, DM], dt.bfloat16, kind="ExternalInput")
    p2w1_d = nc.dram_tensor("p2w1", [BN, DM], dt.bfloat16, kind="ExternalInput")
    p2w2_d = nc.dram_tensor("p2w2", [DM, DM], dt.bfloat16, kind="ExternalInput")
    mb_d = nc.dram_tensor("mbneg", [P, P], dt.bfloat16, kind="ExternalInput")
    id4_d = nc.dram_tensor("id4", [P, 512], dt.bfloat16, kind="ExternalInput")
    hm4_d = nc.dram_tensor("hm4", [P, 512], dt.bfloat16, kind="ExternalInput")
    gind_d = nc.dram_tensor("gind", [P, GRP], dt.float32, kind="ExternalInput")
    wcls_d = nc.dram_tensor("wcls4", [P, ST * DM], dt.float32, kind="ExternalInput")
    ident_d = nc.dram_tensor("ident", [P, P], dt.bfloat16, kind="ExternalInput")
    opt_d = {}
    for nm, shp, cond in [
        ("kqb", [P, 4], flags["qkb"]),
        ("cvb", [P, 264], flags["cv"]),
        ("p1b1", [P, 2], flags["p1b1"]), ("p2b1", [P, 2], flags["p2b1"]),
        ("p1b2b", [P, DM], flags["p1b2"]), ("p2b2b", [P, DM], flags["p2b2"]),
        ("p1gb", [P, DM], flags["p1aff"]), ("p1bb", [P, DM], flags["p1aff"]),
        ("c1gb", [P, DM], flags["c1aff"]), ("c1bb", [P, DM], flags["c1aff"]),
        ("c2gb", [P, DM], flags["c2aff"]), ("c2bb", [P, DM], flags["c2aff"]),
    ]:
        if cond:
            opt_d[nm] = nc.dram_tensor(nm, shp, dt.float32, kind="ExternalInput")
    outp = nc.dram_tensor("outp", [GRP, n_sub], dt.float32, kind="ExternalOutput")

    with tile.TileContext(nc) as tc:
        with contextlib.ExitStack() as ctx:
            singles = ctx.enter_context(tc.tile_pool(name="singles", bufs=1))
            io = ctx.enter_context(tc.tile_pool(name="io", bufs=8))
            work = ctx.enter_context(tc.tile_pool(name="work", bufs=6))

            def load(d, shape, dtp):
                t = singles.tile(shape, dtp, name=d.name + "_sb")
                nc.sync.dma_start(t[:], d[:, :])
                return t

            idx_sb = load(idxc, [P, n_sub], dt.int32)
            npm_sb = load(npmc, [P, n_sub], dt.float32)
            wkq = load(wkq_d, [BN, 512], dt.bfloat16)
            wv = load(wv_d, [BN, 256], dt.bfloat16)
            mb_sb = load(mb_d, [P, P], dt.bfloat16)
            id4_sb = load(id4_d, [P, 512], dt.bfloat16)
            hm4_sb = load(hm4_d, [P, 512], dt.bfloat16)
            gind_sb = load(gind_d, [P, GRP], dt.float32)
            wcls_sb = load(wcls_d, [P, ST * DM], dt.float32)
            ident = load(ident_d, [P, P], dt.bfloat16)
            wfc1, p1w1, p1w2, p2w2 = ([None, None] for _ in range(4))
            for k in range(2):
                for nm, arr, d in (("wfc1", wfc1, wfc1_d), ("p1w1", p1w1, p1w1_d),
                                   ("p1w2", p1w2, p1w2_d), ("p2w2", p2w2, p2w2_d)):
                    arr[k] = singles.tile([P, DM], dt.bfloat16, name=f"{nm}_{k}")
                    nc.sync.dma_start(arr[k][:], d[k * P:(k + 1) * P, :])
            p2w1 = load(p2w1_d, [BN, DM], dt.bfloat16)
            osb = {nm: load(d, d.shape, dt.float32) for nm, d in opt_d.items()}

            res = singles.tile([GRP, 2 * n_sub], dt.float32, name="res")
            cmagic = singles.tile([P, 2 * ST], dt.int32, name="cmagic")
            nc.vector.memset(cmagic[:], 0x5F3759DF)

            # block staging: [nT | emT] and ctx, per subtile 256 bf16 cols
            nem_stage = singles.tile([P, blk * 256], dt.bfloat16, name="nem_stage")
            ctx_stage = singles.tile([P, blk * 256], dt.bfloat16, name="ctx_stage")

            def rsqrt_dve(out_ap, var_ap, n):
                """out = 1/sqrt(var+eps) on DVE only (no scalar act tables):
                Quake bit-trick seed + 2 Newton iterations."""
                ve = work.tile([P, 2 * ST], dt.float32, tag="rsq_ve", name="ve")
                vea = ve[:, 0:n]
                nc.vector.tensor_scalar_add(vea, var_ap, EPS)
                shi = work.tile([P, 2 * ST], dt.int32, tag="rsq_sh", name="shi")
                nc.vector.tensor_scalar(
                    out=shi[:, 0:n], in0=vea.bitcast(dt.int32), scalar1=1,
                    scalar2=None, op0=OP.logical_shift_right)
                z = work.tile([P, 2 * ST], dt.float32, tag="rsq_z", name="z")
                nc.vector.tensor_tensor(out=z[:, 0:n].bitcast(dt.int32),
                                        in0=cmagic[:, 0:n], in1=shi[:, 0:n],
                                        op=OP.subtract)
                t = work.tile([P, 2 * ST], dt.float32, tag="rsq_t", name="t")
                for _ in range(1):
                    nc.vector.tensor_tensor(out=t[:, 0:n], in0=vea,
                                            in1=z[:, 0:n], op=OP.mult)
                    nc.vector.tensor_tensor(out=t[:, 0:n], in0=t[:, 0:n],
                                            in1=z[:, 0:n], op=OP.mult)
                    nc.vector.tensor_scalar(out=t[:, 0:n], in0=t[:, 0:n],
                                            scalar1=-0.5, scalar2=1.5,
                                            op0=OP.mult, op1=OP.add)
                    nc.vector.tensor_tensor(out=z[:, 0:n], in0=z[:, 0:n],
                                            in1=t[:, 0:n], op=OP.mult)
                nc.vector.tensor_copy(out_ap, z[:, 0:n])

            for b in range(n_sub // blk):
                # ================= phase 1: attention -> ctx/nem staging
                with contextlib.ExitStack() as c1:
                    # mix bank: [0:128 fp32-cols = ne-transpose (bf16 view),
                    #            128:384 = v] ; kq and ca share one tag
                    mixp = c1.enter_context(
                        tc.tile_pool(name="mixp", bufs=2, space="PSUM"))
                    kqca = c1.enter_context(
                        tc.tile_pool(name="kqca", bufs=3, space="PSUM"))
                    sp = c1.enter_context(
                        tc.tile_pool(name="sp", bufs=3, space="PSUM"))
                    for i in range(blk):
                        t = b * blk + i
                        stg = slice(i * 256, (i + 1) * 256)

                        ne = io.tile([P, 256], dt.bfloat16, tag="ne", name="ne")
                        nc.gpsimd.indirect_dma_start(
                            out=ne[:], out_offset=None, in_=tabs_d[:, :],
                            in_offset=IOA(ap=idx_sb[:, t:t + 1], axis=0))

                        mix = mixp.tile([P, 384], dt.float32, tag="mix",
                                        name="mix")
                        ne_ps = mix[:, 0:128].bitcast(dt.bfloat16)
                        nc.tensor.transpose(ne_ps[:, 0:P], ne[:, 0:P], ident[:])
                        nc.tensor.transpose(ne_ps[:, P:256], ne[:, P:256],
                                            ident[:])
                        nc.vector.tensor_copy(nem_stage[:, stg], ne_ps[:])
                        nT = nem_stage[:, i * 256:i * 256 + P]

                        kq_ps = kqca.tile([P, 512], dt.float32, tag="kq",
                                          name="kq_ps")
                        for j in range(4):
                            nc.tensor.matmul(kq_ps[:, j * P:(j + 1) * P],
                                             lhsT=wkq[:, j * P:(j + 1) * P],
                                             rhs=nT)
                        kqT = work.tile([P, 512], dt.bfloat16, tag="kqT",
                                        name="kqT")
                        if flags["qkb"]:
                            for j in range(4):
                                nc.scalar.activation(
                                    kqT[:, j * P:(j + 1) * P],
                                    kq_ps[:, j * P:(j + 1) * P],
                                    AF.Identity, bias=osb["kqb"][:, j:j + 1])
                        else:
                            nc.scalar.activation(kqT[:], kq_ps[:], AF.Copy)

                        v_ps = mix[:, 128:384]
                        nc.tensor.matmul(v_ps, lhsT=nT, rhs=wv[:])
                        v_aug = work.tile([P, 264], dt.bfloat16, tag="v_aug",
                                          name="v_aug")
                        va3 = v_aug[:].rearrange("p (h c) -> p h c", c=33)
                        nc.vector.tensor_copy(
                            va3[:, :, 0:32],
                            v_ps.rearrange("p (h c) -> p h c", c=32))
                        if flags["cv"]:
                            nc.vector.tensor_add(v_aug[:], v_aug[:],
                                                 osb["cvb"][:])
                        nc.vector.memset(va3[:, :, 32:33], 1.0)

                        pt = work.tile([P, 1024], dt.bfloat16, tag="pt",
                                       name="pt")
                        for b2 in range(2):
                            q4m = work.tile([P, 512], dt.bfloat16, tag="q4m",
                                            name="q4m")
                            qv = kqT[:, 256 + b2 * P:256 + (b2 + 1) * P]
                            nc.vector.tensor_tensor(
                                out=q4m[:].rearrange("p (j t) -> p j t", j=4),
                                in0=qv.unsqueeze(1).to_broadcast([P, 4, P]),
                                in1=hm4_sb[:].rearrange("p (j t) -> p j t", j=4),
                                op=OP.mult)
                            s_ps = sp.tile([P, 512], dt.float32, tag="s",
                                           name="s_ps")
                            nc.tensor.matmul(s_ps[:], lhsT=mb_sb[:],
                                             rhs=id4_sb[:], start=True,
                                             stop=False)
                            nc.tensor.matmul(s_ps[:],
                                             lhsT=kqT[:, b2 * P:(b2 + 1) * P],
                                             rhs=q4m[:], start=False, stop=True)
                            nc.scalar.activation(pt[:, b2 * 512:(b2 + 1) * 512],
                                                 s_ps[:], AF.Exp)

                        ca_ps = kqca.tile([P, 512], dt.float32, tag="kq",
                                          name="ca_ps")
                        for h in range(H):
                            nc.tensor.matmul(
                                ca_ps[:, h * 33:(h + 1) * 33],
                                lhsT=pt[:, h * P:(h + 1) * P],
                                rhs=v_aug[:, h * 33:(h + 1) * 33])
                        ca3 = ca_ps[:, 0:264].rearrange("p (h c) -> p h c", c=33)
                        rec = work.tile([P, H], dt.float32, tag="rec",
                                        name="rec")
                        rec3 = rec[:].rearrange("p (h o) -> p h o", o=1)
                        nc.vector.reciprocal(rec3[:], ca3[:, :, 32:33])
                        cs3 = ctx_stage[:, stg].rearrange("p (h c) -> p h c",
                                                          c=32)
                        nc.vector.tensor_tensor(
                            out=cs3[:], in0=ca3[:, :, 0:32],
                            in1=rec3.to_broadcast([P, H, 32]), op=OP.mult)

                # ================= phase 2: MLPs + classifier head
                with contextlib.ExitStack() as c2:
                    # bank packing: mix2 = [ct-transpose (bf16 view) | dinT],
                    # dust = [du | st], hh12 = [h1 | h2] -- one alloc per
                    # subtile per tag, so every tag has >=2 subtiles lookahead
                    mix2 = c2.enter_context(
                        tc.tile_pool(name="mix2", bufs=3, space="PSUM"))
                    dust = c2.enter_context(
                        tc.tile_pool(name="dust", bufs=2, space="PSUM"))
                    hh12 = c2.enter_context(
                        tc.tile_pool(name="hh12", bufs=3, space="PSUM"))
                    pend = []

                    def emit_tail(gss, mv_st, u2_st, st2_st):
                            # ======== supertile tail ========
                            mvv = mv_st[:].rearrange("p (j k) -> p j k", k=2)
                            rstd_st = work.tile([P, 2 * ST], dt.float32,
                                                tag="rstds", name="rstd_st")
                            rsqrt_dve(rstd_st[:], mvv[:, :, 1:2].squeeze(2),
                                      2 * ST)
                            # negmrs[:, 2s] = -mu*ru, [:, 2s+1] = -ms*rs, so the
                            # dyn LN-apply can run on Scalar as Identity(u*r + b)
                            nrstd = work.tile([P, 2 * ST], dt.float32, tag="nrstd",
                                              name="nrstd")
                            nc.vector.tensor_scalar_mul(nrstd[:], rstd_st[:], -1.0)
                            negmrs = work.tile([P, 2 * ST], dt.float32,
                                               tag="negmrs", name="negmrs")
                            nc.vector.tensor_tensor(
                                out=negmrs[:].rearrange("p (j o) -> p j o", o=1),
                                in0=mvv[:, :, 0:1], in1=nrstd[:].rearrange(
                                    "p (j o) -> p j o", o=1), op=OP.mult)

                            dyn_st = work.tile([P, ST * DM], dt.float32, tag="dyn",
                                               name="dyn_st", bufs=2)
                            sta_st = work.tile([P, ST * DM], dt.float32, tag="sta",
                                               name="sta_st", bufs=2)
                            for s in range(ST):
                                dyn_sl = dyn_st[:, s * DM:(s + 1) * DM]
                                nc.scalar.activation(
                                    dyn_sl, u2_st[:, s * DM:(s + 1) * DM],
                                    AF.Identity,
                                    bias=negmrs[:, 2 * s:2 * s + 1],
                                    scale=rstd_st[:, 2 * s:2 * s + 1])
                                if flags["p1aff"]:
                                    nc.vector.tensor_mul(dyn_sl, dyn_sl,
                                                             osb["p1gb"][:])
                                    nc.vector.tensor_add(dyn_sl, dyn_sl,
                                                             osb["p1bb"][:])
                                    nc.vector.tensor_scalar(
                                        out=dyn_sl, in0=dyn_sl,
                                        scalar1=npm_sb[:, gss * ST + s:
                                                           gss * ST + s + 1],
                                        scalar2=2.0, op0=OP.mult, op1=OP.mult)
                                if flags["lnc1"]:
                                    st6c = work.tile([P, 6], dt.float32,
                                                         tag="st6c", name="st6c")
                                    nc.vector.bn_stats(st6c[:], dyn_sl)
                                    mvc = work.tile([P, 2], dt.float32, tag="mvc",
                                                        name="mvc")
                                    nc.vector.bn_aggr(mvc[:], st6c[:])
                                    rstdc = work.tile([P, 1], dt.float32,
                                                          tag="rstdc", name="rstdc")
                                    rsqrt_dve(rstdc[:], mvc[:, 1:2], 1)
                                    nc.vector.tensor_scalar(
                                        out=dyn_sl, in0=dyn_sl,
                                        scalar1=mvc[:, 0:1], scalar2=rstdc[:, 0:1],
                                        op0=OP.subtract, op1=OP.mult)
                                if flags["c1aff"]:
                                    nc.vector.tensor_mul(dyn_sl, dyn_sl,
                                                             osb["c1gb"][:])
                                    nc.vector.tensor_add(dyn_sl, dyn_sl,
                                                             osb["c1bb"][:])

                                sta_sl = sta_st[:, s * DM:(s + 1) * DM]
                                nc.scalar.activation(
                                    sta_sl, st2_st[:, s * DM:(s + 1) * DM],
                                    AF.Identity,
                                    bias=negmrs[:, 2 * s + 1:2 * s + 2],
                                    scale=rstd_st[:, 2 * s + 1:2 * s + 2])
                                if flags["c2aff"]:
                                    nc.vector.tensor_mul(sta_sl, sta_sl,
                                                             osb["c2gb"][:])
                                    nc.vector.tensor_add(sta_sl, sta_sl,
                                                             osb["c2bb"][:])

                            dd_st = work.tile([P, ST * DM], dt.float32, tag="dd",
                                              name="dd_st", bufs=2)
                            nc.gpsimd.tensor_tensor(out=dd_st[:], in0=dyn_st[:],
                                                        in1=sta_st[:], op=OP.subtract)
                            dw_st = work.tile([P, ST * DM], dt.float32, tag="dw",
                                              name="dw_st", bufs=2)
                            nc.gpsimd.tensor_tensor(out=dw_st[:], in0=dd_st[:],
                                                        in1=wcls_sb[:], op=OP.mult)
                            wsq_st = work.tile([P, ST * DM], dt.float32, tag="wsq",
                                               name="wsq_st", bufs=2)
                            nc.gpsimd.tensor_tensor(out=wsq_st[:], in0=dd_st[:],
                                                        in1=dw_st[:], op=OP.mult)
                            logit_st = work.tile([P, ST], dt.float32, tag="lg",
                                                 name="logit_st")
                            nc.vector.tensor_reduce(
                                logit_st[:],
                                wsq_st[:].rearrange("p (s d) -> p s d", d=DM),
                                axis=mybir.AxisListType.X, op=OP.add)

                            # sigmoid(z) = 0.5*(1+tanh(z/2)); npm_sb holds npm/2
                            t_st = work.tile([P, ST], dt.float32, tag="tst",
                                             name="t_st")
                            nc.scalar.activation(t_st[:], logit_st[:], AF.Tanh,
                                                 bias=0.5 * bcls, scale=0.5)
                            t1_st = work.tile([P, ST], dt.float32, tag="t1st",
                                              name="t1_st")
                            nc.vector.tensor_scalar_add(t1_st[:], t_st[:], 1.0)

                            npm4 = npm_sb[:, gss * ST:(gss + 1) * ST]
                            pn_st = work.tile([P, 2 * ST], dt.float32, tag="pn",
                                              name="pn_st")
                            pnv = pn_st[:].rearrange("p (s k) -> p s k", k=2)
                            nc.vector.tensor_tensor(
                                out=pnv[:, :, 0:1],
                                in0=t1_st[:].rearrange("p (s o) -> p s o", o=1),
                                in1=npm4.rearrange("p (s o) -> p s o", o=1),
                                op=OP.mult)
                            nc.gpsimd.tensor_scalar_mul(
                                pnv[:, :, 1:2],
                                npm4.rearrange("p (s o) -> p s o", o=1), 2.0)

                            agg_ps = dust.tile([GRP, 2 * ST], dt.float32,
                                               tag="ds", name="agg_ps")
                            nc.tensor.matmul(agg_ps[:], lhsT=gind_sb[:],
                                             rhs=pn_st[:])
                            nc.scalar.activation(
                                res[0:GRP, 2 * ST * gss:2 * ST * (gss + 1)],
                                agg_ps[:], AF.Copy)

                    for ss in range(blk // ST):
                        gss = b * (blk // ST) + ss
                        mv_st = work.tile([P, 4 * ST], dt.float32, tag="mv",
                                          name="mv_st", bufs=2)
                        u2_st = work.tile([P, ST * DM], dt.float32, tag="u2",
                                          name="u2_st", bufs=2)
                        st2_st = work.tile([P, ST * DM], dt.float32, tag="st2",
                                           name="st2_st", bufs=2)
                        for s in range(ST):
                            i = ss * ST + s
                            ctxs = ctx_stage[:, i * 256:(i + 1) * 256]
                            emT = nem_stage[:, i * 256 + P:(i + 1) * 256]

                            m2 = mix2.tile([P, 384], dt.float32, tag="m2",
                                           name="m2")
                            ct_ps = m2[:, 0:128].bitcast(dt.bfloat16)
                            nc.tensor.transpose(ct_ps[:, 0:P], ctxs[:, 0:P],
                                                ident[:])
                            nc.tensor.transpose(ct_ps[:, P:256], ctxs[:, P:256],
                                                ident[:])
                            ctxT = work.tile([P, 256], dt.bfloat16, tag="ctxT",
                                             name="ctxT")
                            nc.vector.tensor_copy(ctxT[:], ct_ps[:])

                            ds = dust.tile([P, 512], dt.float32, tag="ds",
                                           name="ds")
                            du_ps = ds[:, 0:256]
                            nc.tensor.matmul(du_ps, lhsT=ctxT[:, 0:P],
                                             rhs=wfc1[0][:], start=True,
                                             stop=False)
                            nc.tensor.matmul(du_ps, lhsT=ctxT[:, P:256],
                                             rhs=wfc1[1][:], start=False,
                                             stop=True)

                            # flipped fc1: dinT computed weight-stationary,
                            # no transpose of dyn_in needed
                            dinT_ps = m2[:, 128:384]
                            for f in range(2):
                                for k in range(2):
                                    nc.tensor.matmul(
                                        dinT_ps[:, f * P:(f + 1) * P],
                                        lhsT=wfc1[k][:, f * P:(f + 1) * P],
                                        rhs=ctxT[:, k * P:(k + 1) * P],
                                        start=(k == 0), stop=(k == 1))
                            dinT = work.tile([P, 256], dt.bfloat16, tag="dinT",
                                             name="dinT")
                            nc.vector.tensor_copy(dinT[:], dinT_ps)

                            hh = hh12.tile([P, 512], dt.float32, tag="hh",
                                           name="hh")
                            h1_ps = hh[:, 0:256]
                            for f in range(2):
                                for k in range(2):
                                    nc.tensor.matmul(
                                        h1_ps[:, f * P:(f + 1) * P],
                                        lhsT=p1w1[k][:, f * P:(f + 1) * P],
                                        rhs=dinT[:, k * P:(k + 1) * P],
                                        start=(k == 0), stop=(k == 1))
                            h1T = work.tile([P, DM], dt.bfloat16, tag="h1T",
                                            name="h1T")
                            if flags["p1b1"]:
                                for f in range(2):
                                    nc.scalar.activation(
                                        h1T[:, f * P:(f + 1) * P],
                                        h1_ps[:, f * P:(f + 1) * P], AF.Tanh,
                                        bias=osb["p1b1"][:, f:f + 1])
                            else:
                                nc.scalar.activation(h1T[:], h1_ps[:], AF.Tanh)

                            nc.tensor.matmul(du_ps, lhsT=h1T[:, 0:P],
                                             rhs=p1w2[0][:], start=False,
                                             stop=False, skip_group_check=True)
                            nc.tensor.matmul(du_ps, lhsT=h1T[:, P:256],
                                             rhs=p1w2[1][:], start=False,
                                             stop=True, skip_group_check=True)
                            if flags["p1b2"]:
                                nc.vector.tensor_add(du_ps, du_ps,
                                                     osb["p1b2b"][:])
                            u_sl = u2_st[:, s * DM:(s + 1) * DM]
                            st6a = work.tile([P, 6], dt.float32, tag="st6a",
                                             name="st6a")
                            nc.vector.bn_stats(st6a[:], du_ps)
                            nc.scalar.activation(u_sl, du_ps, AF.Copy)
                            nc.vector.bn_aggr(mv_st[:, 4 * s:4 * s + 2],
                                              st6a[:])

                            h2_ps = hh[:, 256:512]
                            for f in range(2):
                                nc.tensor.matmul(h2_ps[:, f * P:(f + 1) * P],
                                                 lhsT=p2w1[:, f * P:(f + 1) * P],
                                                 rhs=emT)
                            h2T = work.tile([P, DM], dt.bfloat16, tag="h2T",
                                            name="h2T")
                            if flags["p2b1"]:
                                for f in range(2):
                                    nc.scalar.activation(
                                        h2T[:, f * P:(f + 1) * P],
                                        h2_ps[:, f * P:(f + 1) * P], AF.Tanh,
                                        bias=osb["p2b1"][:, f:f + 1])
                            else:
                                nc.scalar.activation(h2T[:], h2_ps[:], AF.Tanh)

                            st_ps = ds[:, 256:512]
                            nc.tensor.matmul(st_ps, lhsT=h2T[:, 0:P],
                                             rhs=p2w2[0][:], start=True,
                                             stop=False)
                            nc.tensor.matmul(st_ps, lhsT=h2T[:, P:256],
                                             rhs=p2w2[1][:], start=False,
                                             stop=True)
                            if flags["p2b2"]:
                                nc.vector.tensor_add(st_ps, st_ps,
                                                     osb["p2b2b"][:])
                            st_sl = st2_st[:, s * DM:(s + 1) * DM]
                            st6b = work.tile([P, 6], dt.float32, tag="st6b",
                                             name="st6b")
                            nc.vector.bn_stats(st6b[:], st_ps)
                            nc.scalar.activation(st_sl, st_ps, AF.Copy)
                            nc.vector.bn_aggr(mv_st[:, 4 * s + 2:4 * s + 4],
                                              st6b[:])

                        # defer this supertile's tail by one supertile so the
                        # serial gpsimd dd/dw/wsq chain never head-of-line
                        # blocks the vector queue at tensor_reduce
                        pend.append((gss, mv_st, u2_st, st2_st))
                        if len(pend) > 1:
                            emit_tail(*pend.pop(0))


                    while pend:
                        emit_tail(*pend.pop(0))

            # ---- final divide + store
            r3 = res[:].rearrange("p (t k) -> p t k", k=2)
            rn = work.tile([GRP, n_sub], dt.float32, tag="rn", name="rn")
            rn3 = rn[:].rearrange("p (t o) -> p t o", o=1)
            nc.vector.reciprocal(rn3[:], r3[:, :, 1:2])
            orow = work.tile([GRP, n_sub], dt.float32, tag="orow", name="orow")
            orow3 = orow[:].rearrange("p (t o) -> p t o", o=1)
            nc.vector.tensor_tensor(out=orow3[:], in0=r3[:, :, 0:1], in1=rn3[:],
                                    op=OP.mult)
            nc.sync.dma_start(outp[:, :], orow[:])

    nc.finalize()
    return nc


# ----------------------------------------------------------------- entry
_NC_CACHE = {}


def kernel(**inputs):
    _install_ntff_hook()
    from concourse.bass_utils import run_bass_kernel_spmd

    n_sub = int(os.environ.get("KBENCH_NSUB", NSUB_FULL))
    consts = _prep_consts(inputs)
    flags = consts.pop("_flags")
    bcls = consts.pop("_bcls")

    stage = int(os.environ.get("KBENCH_STAGE", "8"))
    key = (n_sub, stage, tuple(sorted(flags.items())))
    if key not in _NC_CACHE:
        _NC_CACHE[key] = build_nc(flags, bcls, n_sub, stage)
    nc = _NC_CACHE[key]

    x = np.asarray(inputs["x"]).astype(np.int32)
    in_maps = []
    for c in range(NCORES):
        xc = x[c * BC:(c + 1) * BC].reshape(-1)          # [16384]
        idxc = np.ascontiguousarray(
            xc[:n_sub * P].reshape(n_sub, P).T)          # [128, n_sub]
        m = {"idxc": idxc, "npmc": 0.5 * (idxc != 0).astype(np.float32)}
        m.update(consts)
        in_maps.append(m)

    trace = bool(int(os.environ.get("KBENCH_TRACE", "0")))
    res = run_bass_kernel_spmd(nc, in_maps, core_ids=list(range(NCORES)),
                               trace=trace)
    kernel._last_results = res

    out = np.zeros((B, 1), np.float32)
    for c in range(NCORES):
        oc = res.results[c]["outp"]                      # [8, n_sub]
        out[c * BC:c * BC + n_sub * GRP, 0] = oc.T.reshape(-1)
    return out


# revision 56
# speedup vs baseline: 1.2349x; 1.2261x over previous
"""Self-contained Trainium2 Bass kernel for nn_Classifier_79929341379065.

kernel(**inputs) takes FULL unsharded inputs (as produced by
reference.setup_inputs()) and returns the FULL [B, 1] float32 output.
Internally: pure data parallel over 8 NeuronCores (batch dim of x),
weights replicated.

Hardcoded shapes: B=8192, L=16, H=8, DK=DV=32, DM=256, BN=128, V=50000.
Per core: 1024 batches = 16384 tokens = 128 subtiles of 128 tokens
(each subtile = 8 attention groups of L=16).

Two-phase blocked pipeline (32 subtiles per block) so each phase gets
all 8 PSUM banks and a short dependency chain:
  phase 1 (attention): gather [n|em] rows from merged table [V,256]
    (col 0:128 = LN-normalized node_emb bf16, 128:256 = raw row with
    row 0 zeroed); PE-transpose to [nT|emT] staging; compact k|q
    projection in one PSUM bank; head-masked q4 built on DVE via
    broadcast*mask; S^T = one K=128 N=512 matmul per 4 heads with the
    additive -1e9 group/diag mask PRE-loaded into PSUM by a
    (mask, tiled-identity) matmul (start=True) so exp(PSUM) is already
    masked; PV with ones-augmented V gives [ctx~|den]; ctx staged bf16.
  phase 2 (MLPs+head): ctx/din transposes, fc1 + residual kept in PSUM
    accumulation, tanh MLPs, static path from emT; LN via bn_stats +
    Quake bit-trick rsqrt on DVE (no scalar act-table switches ever:
    scalar only runs Copy/Identity/Exp/Tanh from one table);
    logit = sum((dyn-sta)^2*wcls) with the elementwise chain on GpSimd;
    sigmoid(z) = 0.5*(1+tanh(z/2)) with the 0.5 folded into host-side
    npm/2; one [8,8] per-group aggregation matmul; single final divide.
"""

import os
import sys
import types

import numpy as np

# ---------------------------------------------------------------- constants
B, L = 8192, 16
H, DK, DV = 8, 32, 32
DM, BN, V = 256, 128, 50000
NCORES = 8
P = 128
BC = B // NCORES                  # batches per core (1024)
TOKC = BC * L                     # tokens per core (16384)
NSUB_FULL = TOKC // P             # subtiles per core (128)
GRP = P // L                      # groups per subtile (8)
ST = 8                            # subtiles per supertile
SCL = 1.0 / np.sqrt(float(DK))
EPS = 1e-5
MNEG = -1e9                       # additive mask value (exp -> 0)


def _install_ntff_hook():
    """Register the axon NTFF profiling hook if the image's antenv lacks it,
    so run_bass_kernel_spmd(trace=True) works in this container."""
    try:
        import antenv.axon_hooks  # noqa: F401
        return
    except ImportError:
        pass
    try:
        from trn_agent_boot.trn_boot import _ntff_profile_via_ctypes
        hook = _ntff_profile_via_ctypes("/opt/axon/libaxon_pjrt.so")
    except Exception:
        hook = None
    m = types.ModuleType("antenv.axon_hooks")
    m.get_axon_ntff_profile_hook = lambda: hook
    m.set_axon_ntff_profile_hook = lambda h: None
    sys.modules["antenv.axon_hooks"] = m


def _bf16(a):
    import ml_dtypes
    return np.ascontiguousarray(a.astype(ml_dtypes.bfloat16))


def _triv(g, b):
    return bool(np.allclose(g, 1.0, atol=1e-12) and np.allclose(b, 0.0, atol=1e-12))


# ------------------------------------------------------------- host weights
def _prep_consts(w):
    """Fold LN affines into projection weights; build device const arrays."""
    c = {}
    f32 = np.float32

    wq_eff = (np.asarray(w["Wq"], f32) * np.asarray(w["ln1_g"], f32)[None, :]) * SCL
    wk_eff = np.asarray(w["Wk"], f32) * np.asarray(w["ln2_g"], f32)[None, :]
    wv_eff = np.asarray(w["Wv"], f32) * np.asarray(w["ln3_g"], f32)[None, :]
    cq = (np.asarray(w["ln1_b"], f32) @ np.asarray(w["Wq"], f32).T) * SCL
    ck = np.asarray(w["ln2_b"], f32) @ np.asarray(w["Wk"], f32).T
    cv = np.asarray(w["ln3_b"], f32) @ np.asarray(w["Wv"], f32).T

    # merged compact k|q weights: [BN, 512] = [wk.T blk0|blk1 | wq.T blk0|blk1]
    c["wkq"] = _bf16(np.concatenate([wk_eff.T, wq_eff.T], axis=1))  # [BN, 512]

    c["wv"] = _bf16(wv_eff.T)                                # [BN, 256]

    c["wfc1"] = _bf16(np.asarray(w["Wfc1"], f32).T)          # [HDV, DM] rhs
    c["p1w1"] = _bf16(np.asarray(w["p1_w1"], f32).T)         # [DM, DM] lhsT
    c["p1w2"] = _bf16(np.asarray(w["p1_w2"], f32).T)         # [DM, DM] rhs
    c["p2w1"] = _bf16(np.asarray(w["p2_w1"], f32).T)         # [BN, DM] lhsT
    c["p2w2"] = _bf16(np.asarray(w["p2_w2"], f32).T)         # [DM, DM] rhs

    # merged gather table: [V, 256] = [LN-normalized | raw with row0 zeroed]
    tab = np.asarray(w["node_emb"], f32)
    m = tab.mean(axis=1, keepdims=True)
    v = ((tab - m) ** 2).mean(axis=1, keepdims=True)
    tabn = (tab - m) / np.sqrt(v + EPS)
    tabe = tab.copy()
    tabe[0, :] = 0.0
    c["tabs"] = _bf16(np.concatenate([tabn, tabe], axis=1))  # [V, 256]

    # additive mask: -1e9 where cross-group or diagonal, else 0 (symmetric)
    mb = np.full((P, P), MNEG, f32)
    for g in range(GRP):
        mb[g * L:(g + 1) * L, g * L:(g + 1) * L] = 0.0
    mb[np.eye(P, dtype=bool)] = MNEG
    c["mbneg"] = _bf16(mb)                                   # [128,128]
    c["id4"] = _bf16(np.tile(np.eye(P, dtype=f32), (1, 4)))  # [128,512]
    # 0/1 head mask for building q4 from compact q: hm[f, hh*128+t]=(f//32==hh)
    hm = np.zeros((P, 512), f32)
    for hh in range(4):
        hm[hh * 32:(hh + 1) * 32, hh * P:(hh + 1) * P] = 1.0
    c["hm4"] = _bf16(hm)                                     # [128,512]

    gind = np.zeros((P, GRP), f32)
    for g in range(GRP):
        gind[g * L:(g + 1) * L, g] = 1.0
    c["gind"] = gind

    wcls_row = np.asarray(w["Wcls"], f32).reshape(1, DM)
    c["wcls4"] = np.ascontiguousarray(
        np.broadcast_to(np.tile(wcls_row, (1, ST)), (P, ST * DM)))  # [128,1024]
    c["ident"] = _bf16(np.eye(P, dtype=f32))

    flags = {
        "qkb": not (np.allclose(cq, 0.0) and np.allclose(ck, 0.0)),
        "cv": not np.allclose(cv, 0.0),
        "p1b1": not np.allclose(w["p1_b1"], 0.0),
        "p2b1": not np.allclose(w["p2_b1"], 0.0),
        "p1b2": not np.allclose(w["p1_b2"], 0.0),
        "p2b2": not np.allclose(w["p2_b2"], 0.0),
        "p1aff": not _triv(w["p1_lng"], w["p1_lnb"]),
        "c1aff": not _triv(w["lnc1_g"], w["lnc1_b"]),
        "c2aff": not _triv(w["lnc2_g"], w["lnc2_b"]),
    }
    flags["lnc1"] = flags["p1aff"]

    if flags["qkb"]:
        # per-128-block biases for the kq copy: cols [k0,k1,q0,q1]
        kqb = np.zeros((P, 4), f32)
        kqb[:, 0], kqb[:, 1] = ck[0:128], ck[128:256]
        kqb[:, 2], kqb[:, 3] = cq[0:128], cq[128:256]
        c["kqb"] = kqb
    if flags["cv"]:
        cvb = np.zeros((P, H * 33), f32)
        for h in range(H):
            cvb[:, h * 33:h * 33 + 32] = np.broadcast_to(
                cv[h * 32:(h + 1) * 32][None, :], (P, 32))
        c["cvb"] = cvb
    if flags["p1b1"]:
        c["p1b1"] = np.stack([np.asarray(w["p1_b1"], f32)[0:128],
                              np.asarray(w["p1_b1"], f32)[128:256]], 1)
    if flags["p2b1"]:
        c["p2b1"] = np.stack([np.asarray(w["p2_b1"], f32)[0:128],
                              np.asarray(w["p2_b1"], f32)[128:256]], 1)
    if flags["p1b2"]:
        c["p1b2b"] = np.broadcast_to(
            np.asarray(w["p1_b2"], f32)[None, :], (P, DM)).copy()
    if flags["p2b2"]:
        c["p2b2b"] = np.broadcast_to(
            np.asarray(w["p2_b2"], f32)[None, :], (P, DM)).copy()
    for nm, gk, bk in (("p1", "p1_lng", "p1_lnb"), ("c1", "lnc1_g", "lnc1_b"),
                       ("c2", "lnc2_g", "lnc2_b")):
        if flags[nm + "aff"]:
            c[nm + "gb"] = np.broadcast_to(
                np.asarray(w[gk], f32)[None, :], (P, DM)).copy()
            c[nm + "bb"] = np.broadcast_to(
                np.asarray(w[bk], f32)[None, :], (P, DM)).copy()

    c["_bcls"] = float(np.asarray(w["bcls"]).reshape(-1)[0])
    c["_flags"] = flags
    return c


# ------------------------------------------------------------ device program
def build_nc(flags, bcls, n_sub, stage=8):
    import contextlib

    import concourse.bacc as bacc
    import concourse.tile as tile
    import concourse.mybir as mybir
    from concourse import bass

    dt = mybir.dt
    AF = mybir.ActivationFunctionType
    OP = mybir.AluOpType
    IOA = bass.IndirectOffsetOnAxis
    BLK = 32                       # subtiles per phase block
    assert n_sub % ST == 0
    blk = min(BLK, n_sub)
    assert n_sub % blk == 0 and blk % ST == 0

    nc = bacc.Bacc()

    # ---- dram tensors
    idxc = nc.dram_tensor("idxc", [P, n_sub], dt.int32, kind="ExternalInput")
    npmc = nc.dram_tensor("npmc", [P, n_sub], dt.float32, kind="ExternalInput")
    tabs_d = nc.dram_tensor("tabs", [V, 2 * BN], dt.bfloat16, kind="ExternalInput")
    wkq_d = nc.dram_tensor("wkq", [BN, 512], dt.bfloat16, kind="ExternalInput")
    wv_d = nc.dram_tensor("wv", [BN, 256], dt.bfloat16, kind="ExternalInput")
    wfc1_d = nc.dram_tensor("wfc1", [DM, DM], dt.bfloat16, kind="ExternalInput")
    p1w1_d = nc.dram_tensor("p1w1", [DM, DM], dt.bfloat16, kind="ExternalInput")
    p1w2_d = nc.dram_tensor("p1w2", [DM, DM], dt.bfloat16, kind="ExternalInput")
    p2w1_d = nc.dram_tensor("p2w1", [BN, DM], dt.bfloat16, kind="ExternalInput")
    p2w2_d = nc.dram_tensor("p2w2", [DM, DM], dt.bfloat16, kind="ExternalInput")
    mb_d = nc.dram_tensor("mbneg", [P, P], dt.bfloat16, kind="ExternalInput")
    id4_d = nc.dram_tensor("id4", [P, 512], dt.bfloat16, kind="ExternalInput")
    hm4_d = nc.dram_tensor("hm4", [P, 512], dt.bfloat16, kind="ExternalInput")
    gind_d = nc.dram_tensor("gind", [P, GRP], dt.float32, kind="ExternalInput")
    wcls_d = nc.dram_tensor("wcls4", [P, ST * DM], dt.float32, kind="ExternalInput")
    ident_d = nc.dram_tensor("ident", [P, P], dt.bfloat16, kind="ExternalInput")
    opt_d = {}
    for nm, shp, cond in [
        ("kqb", [P, 4], flags["qkb"]),
        ("cvb", [P, 264], flags["cv"]),
        ("p1b1", [P, 2], flags["p1b1"]), ("p2b1", [P, 2], flags["p2b1"]),
        ("p1b2b", [P, DM], flags["p1b2"]), ("p2b2b", [P, DM], flags["p2b2"]),
        ("p1gb", [P, DM], flags["p1aff"]), ("p1bb", [P, DM], flags["p1aff"]),
        ("c1gb", [P, DM], flags["c1aff"]), ("c1bb", [P, DM], flags["c1aff"]),
        ("c2gb", [P, DM], flags["c2aff"]), ("c2bb", [P, DM], flags["c2aff"]),
    ]:
        if cond:
            opt_d[nm] = nc.dram_tensor(nm, shp, dt.float32, kind="ExternalInput")
    outp = nc.dram_tensor("outp", [GRP, n_sub], dt.float32, kind="ExternalOutput")

    with tile.TileContext(nc) as tc:
        with contextlib.ExitStack() as ctx:
            singles = ctx.enter_context(tc.tile_pool(name="singles", bufs=1))
            io = ctx.enter_context(tc.tile_pool(name="io", bufs=8))
            work = ctx.enter_context(tc.tile_pool(name="work", bufs=6))

            def load(d, shape, dtp):
                t = singles.tile(shape, dtp, name=d.name + "_sb")
                nc.sync.dma_start(t[:], d[:, :])
                return t

            idx_sb = load(idxc, [P, n_sub], dt.int32)
            npm_sb = load(npmc, [P, n_sub], dt.float32)
            wkq = load(wkq_d, [BN, 512], dt.bfloat16)
            wv = load(wv_d, [BN, 256], dt.bfloat16)
            mb_sb = load(mb_d, [P, P], dt.bfloat16)
            id4_sb = load(id4_d, [P, 512], dt.bfloat16)
            hm4_sb = load(hm4_d, [P, 512], dt.bfloat16)
            gind_sb = load(gind_d, [P, GRP], dt.float32)
            wcls_sb = load(wcls_d, [P, ST * DM], dt.float32)
            ident = load(ident_d, [P, P], dt.bfloat16)
            wfc1, p1w1, p1w2, p2w2 = ([None, None] for _ in range(4))
            for k in range(2):
                for nm, arr, d in (("wfc1", wfc1, wfc1_d), ("p1w1", p1w1, p1w1_d),
                                   ("p1w2", p1w2, p1w2_d), ("p2w2", p2w2, p2w2_d)):
                    arr[k] = singles.tile([P, DM], dt.bfloat16, name=f"{nm}_{k}")
                    nc.sync.dma_start(arr[k][:], d[k * P:(k + 1) * P, :])
            p2w1 = load(p2w1_d, [BN, DM], dt.bfloat16)
            osb = {nm: load(d, d.shape, dt.float32) for nm, d in opt_d.items()}

            res = singles.tile([GRP, 2 * n_sub], dt.float32, name="res")
            cmagic = singles.tile([P, 2 * ST], dt.int32, name="cmagic")
            nc.vector.memset(cmagic[:], 0x5F3759DF)

            # block staging: [nT | emT] and ctx, per subtile 256 bf16 cols
            nem_stage = singles.tile([P, blk * 256], dt.bfloat16, name="nem_stage")
            ctx_stage = singles.tile([P, blk * 256], dt.bfloat16, name="ctx_stage")

            def rsqrt_dve(out_ap, var_ap, n):
                """out = 1/sqrt(var+eps) on DVE only (no scalar act tables):
                Quake bit-trick seed + 2 Newton iterations."""
                ve = work.tile([P, 2 * ST], dt.float32, tag="rsq_ve", name="ve")
                vea = ve[:, 0:n]
                nc.vector.tensor_scalar_add(vea, var_ap, EPS)
                shi = work.tile([P, 2 * ST], dt.int32, tag="rsq_sh", name="shi")
                nc.vector.tensor_scalar(
                    out=shi[:, 0:n], in0=vea.bitcast(dt.int32), scalar1=1,
                    scalar2=None, op0=OP.logical_shift_right)
                z = work.tile([P, 2 * ST], dt.float32, tag="rsq_z", name="z")
                nc.vector.tensor_tensor(out=z[:, 0:n].bitcast(dt.int32),
                                        in0=cmagic[:, 0:n], in1=shi[:, 0:n],
                                        op=OP.subtract)
                t = work.tile([P, 2 * ST], dt.float32, tag="rsq_t", name="t")
                for _ in range(1):
                    nc.vector.tensor_tensor(out=t[:, 0:n], in0=vea,
                                            in1=z[:, 0:n], op=OP.mult)
                    nc.vector.tensor_tensor(out=t[:, 0:n], in0=t[:, 0:n],
                                            in1=z[:, 0:n], op=OP.mult)
                    nc.vector.tensor_scalar(out=t[:, 0:n], in0=t[:, 0:n],
                                            scalar1=-0.5, scalar2=1.5,
                                            op0=OP.mult, op1=OP.add)
                    nc.vector.tensor_tensor(out=z[:, 0:n], in0=z[:, 0:n],
                                            in1=t[:, 0:n], op=OP.mult)
                nc.vector.tensor_copy(out_ap, z[:, 0:n])

            for b in range(n_sub // blk):
                # ================= phase 1: attention -> ctx/nem staging
                with contextlib.ExitStack() as c1:
                    # mix bank: [0:128 fp32-cols = ne-transpose (bf16 view),
                    #            128:384 = v] ; kq and ca share one tag
                    mixp = c1.enter_context(
                        tc.tile_pool(name="mixp", bufs=2, space="PSUM"))
                    kqca = c1.enter_context(
                        tc.tile_pool(name="kqca", bufs=3, space="PSUM"))
                    sp = c1.enter_context(
                        tc.tile_pool(name="sp", bufs=3, space="PSUM"))
                    for i in range(blk):
                        t = b * blk + i
                        stg = slice(i * 256, (i + 1) * 256)

                        ne = io.tile([P, 256], dt.bfloat16, tag="ne", name="ne")
                        nc.gpsimd.indirect_dma_start(
                            out=ne[:], out_offset=None, in_=tabs_d[:, :],
                            in_offset=IOA(ap=idx_sb[:, t:t + 1], axis=0))

                        mix = mixp.tile([P, 384], dt.float32, tag="mix",
                                        name="mix")
                        ne_ps = mix[:, 0:128].bitcast(dt.bfloat16)
                        nc.tensor.transpose(ne_ps[:, 0:P], ne[:, 0:P], ident[:])
                        nc.tensor.transpose(ne_ps[:, P:256], ne[:, P:256],
                                            ident[:])
                        nc.vector.tensor_copy(nem_stage[:, stg], ne_ps[:])
                        nT = nem_stage[:, i * 256:i * 256 + P]

                        kq_ps = kqca.tile([P, 512], dt.float32, tag="kq",
                                          name="kq_ps")
                        for j in range(4):
                            nc.tensor.matmul(kq_ps[:, j * P:(j + 1) * P],
                                             lhsT=wkq[:, j * P:(j + 1) * P],
                                             rhs=nT)
                        kqT = work.tile([P, 512], dt.bfloat16, tag="kqT",
                                        name="kqT")
                        if flags["qkb"]:
                            for j in range(4):
                                nc.scalar.activation(
                                    kqT[:, j * P:(j + 1) * P],
                                    kq_ps[:, j * P:(j + 1) * P],
                                    AF.Identity, bias=osb["kqb"][:, j:j + 1])
                        else:
                            nc.scalar.activation(kqT[:], kq_ps[:], AF.Copy)

                        v_ps = mix[:, 128:384]
                        nc.tensor.matmul(v_ps, lhsT=nT, rhs=wv[:])
                        v_aug = work.tile([P, 264], dt.bfloat16, tag="v_aug",
                                          name="v_aug")
                        va3 = v_aug[:].rearrange("p (h c) -> p h c", c=33)
                        nc.vector.tensor_copy(
                            va3[:, :, 0:32],
                            v_ps.rearrange("p (h c) -> p h c", c=32))
                        if flags["cv"]:
                            nc.vector.tensor_add(v_aug[:], v_aug[:],
                                                 osb["cvb"][:])
                        nc.vector.memset(va3[:, :, 32:33], 1.0)

                        pt = work.tile([P, 1024], dt.bfloat16, tag="pt",
                                       name="pt")
                        for b2 in range(2):
                            q4m = work.tile([P, 512], dt.bfloat16, tag="q4m",
                                            name="q4m")
                            qv = kqT[:, 256 + b2 * P:256 + (b2 + 1) * P]
                            nc.vector.tensor_tensor(
                                out=q4m[:].rearrange("p (j t) -> p j t", j=4),
                                in0=qv.unsqueeze(1).to_broadcast([P, 4, P]),
                                in1=hm4_sb[:].rearrange("p (j t) -> p j t", j=4),
                                op=OP.mult)
                            s_ps = sp.tile([P, 512], dt.float32, tag="s",
                                           name="s_ps")
                            nc.tensor.matmul(s_ps[:], lhsT=mb_sb[:],
                                             rhs=id4_sb[:], start=True,
                                             stop=False)
                            nc.tensor.matmul(s_ps[:],
                                             lhsT=kqT[:, b2 * P:(b2 + 1) * P],
                                             rhs=q4m[:], start=False, stop=True)
                            nc.scalar.activation(pt[:, b2 * 512:(b2 + 1) * 512],
                                                 s_ps[:], AF.Exp)

                        ca_ps = kqca.tile([P, 512], dt.float32, tag="kq",
                                          name="ca_ps")
                        for h in range(H):
                            nc.tensor.matmul(
                                ca_ps[:, h * 33:(h + 1) * 33],
                                lhsT=pt[:, h * P:(h + 1) * P],
                                rhs=v_aug[:, h * 33:(h + 1) * 33])
                        ca3 = ca_ps[:, 0:264].rearrange("p (h c) -> p h c", c=33)
                        rec = work.tile([P, H], dt.float32, tag="rec",
                                        name="rec")
                        rec3 = rec[:].rearrange("p (h o) -> p h o", o=1)
                        nc.vector.reciprocal(rec3[:], ca3[:, :, 32:33])
                        cs3 = ctx_stage[:, stg].rearrange("p (h c) -> p h c",
                                                          c=32)
                        nc.vector.tensor_tensor(
                            out=cs3[:], in0=ca3[:, :, 0:32],
                            in1=rec3.to_broadcast([P, H, 32]), op=OP.mult)

                # ================= phase 2: MLPs + classifier head
                with contextlib.ExitStack() as c2:
                    # bank packing: mix2 = [ct-transpose (bf16 view) | dinT],
                    # dust = [du | st], hh12 = [h1 | h2] -- one alloc per
                    # subtile per tag, so every tag has >=2 subtiles lookahead
                    mix2 = c2.enter_context(
                        tc.tile_pool(name="mix2", bufs=3, space="PSUM"))
                    dust = c2.enter_context(
                        tc.tile_pool(name="dust", bufs=2, space="PSUM"))
                    hh12 = c2.enter_context(
                        tc.tile_pool(name="hh12", bufs=3, space="PSUM"))
                    pend = []

                    def emit_tail(gss, mv_st, u2_st, st2_st):
                            # ======== supertile tail ========
                            mvv = mv_st[:].rearrange("p (j k) -> p j k", k=2)
                            rstd_st = work.tile([P, 2 * ST], dt.float32,
                                                tag="rstds", name="rstd_st")
                            rsqrt_dve(rstd_st[:], mvv[:, :, 1:2].squeeze(2),
                                      2 * ST)
                            # negmrs[:, 2s] = -mu*ru, [:, 2s+1] = -ms*rs, so the
                            # dyn LN-apply can run on Scalar as Identity(u*r + b)
                            nrstd = work.tile([P, 2 * ST], dt.float32, tag="nrstd",
                                              name="nrstd")
                            nc.vector.tensor_scalar_mul(nrstd[:], rstd_st[:], -1.0)
                            negmrs = work.tile([P, 2 * ST], dt.float32,
                                               tag="negmrs", name="negmrs")
                            nc.vector.tensor_tensor(
                                out=negmrs[:].rearrange("p (j o) -> p j o", o=1),
                                in0=mvv[:, :, 0:1], in1=nrstd[:].rearrange(
                                    "p (j o) -> p j o", o=1), op=OP.mult)

                            dyn_st = work.tile([P, ST * DM], dt.float32, tag="dyn",
                                               name="dyn_st", bufs=2)
                            sta_st = work.tile([P, ST * DM], dt.float32, tag="sta",
                                               name="sta_st", bufs=2)
                            for s in range(ST):
                                dyn_sl = dyn_st[:, s * DM:(s + 1) * DM]
                                nc.scalar.activation(
                                    dyn_sl, u2_st[:, s * DM:(s + 1) * DM],
                                    AF.Identity,
                                    bias=negmrs[:, 2 * s:2 * s + 1],
                                    scale=rstd_st[:, 2 * s:2 * s + 1])
                                if flags["p1aff"]:
                                    nc.vector.tensor_mul(dyn_sl, dyn_sl,
                                                             osb["p1gb"][:])
                                    nc.vector.tensor_add(dyn_sl, dyn_sl,
                                                             osb["p1bb"][:])
                                    nc.vector.tensor_scalar(
                                        out=dyn_sl, in0=dyn_sl,
                                        scalar1=npm_sb[:, gss * ST + s:
                                                           gss * ST + s + 1],
                                        scalar2=2.0, op0=OP.mult, op1=OP.mult)
                                if flags["lnc1"]:
                                    st6c = work.tile([P, 6], dt.float32,
                                                         tag="st6c", name="st6c")
                                    nc.vector.bn_stats(st6c[:], dyn_sl)
                                    mvc = work.tile([P, 2], dt.float32, tag="mvc",
                                                        name="mvc")
                                    nc.vector.bn_aggr(mvc[:], st6c[:])
                                    rstdc = work.tile([P, 1], dt.float32,
                                                          tag="rstdc", name="rstdc")
                                    rsqrt_dve(rstdc[:], mvc[:, 1:2], 1)
                                    nc.vector.tensor_scalar(
                                        out=dyn_sl, in0=dyn_sl,
                                        scalar1=mvc[:, 0:1], scalar2=rstdc[:, 0:1],
                                        op0=OP.subtract, op1=OP.mult)
                                if flags["c1aff"]:
                                    nc.vector.tensor_mul(dyn_sl, dyn_sl,
                                                             osb["c1gb"][:])
                                    nc.vector.tensor_add(dyn_sl, dyn_sl,
                                                             osb["c1bb"][:])

                                sta_sl = sta_st[:, s * DM:(s + 1) * DM]
                                nc.scalar.activation(
                                    sta_sl, st2_st[:, s * DM:(s + 1) * DM],
                                    AF.Identity,
                                    bias=negmrs[:, 2 * s + 1:2 * s + 2],
                                    scale=rstd_st[:, 2 * s + 1:2 * s + 2])
                                if flags["c2aff"]:
                                    nc.vector.tensor_mul(sta_sl, sta_sl,
                                                             osb["c2gb"][:])
                                    nc.vector.tensor_add(sta_sl, sta_sl,
                                                             osb["c2bb"][:])

                            dd_st = work.tile([P, ST * DM], dt.float32, tag="dd",
                                              name="dd_st", bufs=2)
                            nc.gpsimd.tensor_tensor(out=dd_st[:], in0=dyn_st[:],
                                                        in1=sta_st[:], op=OP.subtract)
                            dw_st = work.tile([P, ST * DM], dt.float32, tag="dw",
                                              name="dw_st", bufs=1)
                            nc.gpsimd.tensor_tensor(out=dw_st[:], in0=dd_st[:],
                                                        in1=wcls_sb[:], op=OP.mult)
                            wsq_st = work.tile([P, ST * DM], dt.float32, tag="wsq",
                                               name="wsq_st", bufs=1)
                            nc.gpsimd.tensor_tensor(out=wsq_st[:], in0=dd_st[:],
                                                        in1=dw_st[:], op=OP.mult)
                            logit_st = work.tile([P, ST], dt.float32, tag="lg",
                                                 name="logit_st")
                            nc.vector.tensor_reduce(
                                logit_st[:],
                                wsq_st[:].rearrange("p (s d) -> p s d", d=DM),
                                axis=mybir.AxisListType.X, op=OP.add)

                            # sigmoid(z) = 0.5*(1+tanh(z/2)); npm_sb holds npm/2
                            t_st = work.tile([P, ST], dt.float32, tag="tst",
                                             name="t_st")
                            nc.scalar.activation(t_st[:], logit_st[:], AF.Tanh,
                                                 bias=0.5 * bcls, scale=0.5)
                            t1_st = work.tile([P, ST], dt.float32, tag="t1st",
                                              name="t1_st")
                            nc.vector.tensor_scalar_add(t1_st[:], t_st[:], 1.0)

                            npm4 = npm_sb[:, gss * ST:(gss + 1) * ST]
                            pn_st = work.tile([P, 2 * ST], dt.float32, tag="pn",
                                              name="pn_st")
                            pnv = pn_st[:].rearrange("p (s k) -> p s k", k=2)
                            nc.vector.tensor_tensor(
                                out=pnv[:, :, 0:1],
                                in0=t1_st[:].rearrange("p (s o) -> p s o", o=1),
                                in1=npm4.rearrange("p (s o) -> p s o", o=1),
                                op=OP.mult)
                            nc.gpsimd.tensor_scalar_mul(
                                pnv[:, :, 1:2],
                                npm4.rearrange("p (s o) -> p s o", o=1), 2.0)

                            agg_ps = dust.tile([GRP, 2 * ST], dt.float32,
                                               tag="ds", name="agg_ps")
                            nc.tensor.matmul(agg_ps[:], lhsT=gind_sb[:],
                                             rhs=pn_st[:])
                            nc.scalar.activation(
                                res[0:GRP, 2 * ST * gss:2 * ST * (gss + 1)],
                                agg_ps[:], AF.Copy)

                    for ss in range(blk // ST):
                        gss = b * (blk // ST) + ss
                        mv_st = work.tile([P, 4 * ST], dt.float32, tag="mv",
                                          name="mv_st", bufs=2)
                        u2_st = work.tile([P, ST * DM], dt.float32, tag="u2",
                                          name="u2_st", bufs=2)
                        st2_st = work.tile([P, ST * DM], dt.float32, tag="st2",
                                           name="st2_st", bufs=2)
                        for s in range(ST):
                            i = ss * ST + s
                            ctxs = ctx_stage[:, i * 256:(i + 1) * 256]
                            emT = nem_stage[:, i * 256 + P:(i + 1) * 256]

                            m2 = mix2.tile([P, 384], dt.float32, tag="m2",
                                           name="m2")
                            ct_ps = m2[:, 0:128].bitcast(dt.bfloat16)
                            nc.tensor.transpose(ct_ps[:, 0:P], ctxs[:, 0:P],
                                                ident[:])
                            nc.tensor.transpose(ct_ps[:, P:256], ctxs[:, P:256],
                                                ident[:])
                            ctxT = work.tile([P, 256], dt.bfloat16, tag="ctxT",
                                             name="ctxT")
                            nc.vector.tensor_copy(ctxT[:], ct_ps[:])

                            ds = dust.tile([P, 512], dt.float32, tag="ds",
                                           name="ds")
                            du_ps = ds[:, 0:256]
                            nc.tensor.matmul(du_ps, lhsT=ctxT[:, 0:P],
                                             rhs=wfc1[0][:], start=True,
                                             stop=False)
                            nc.tensor.matmul(du_ps, lhsT=ctxT[:, P:256],
                                             rhs=wfc1[1][:], start=False,
                                             stop=True)

                            # flipped fc1: dinT computed weight-stationary,
                            # no transpose of dyn_in needed
                            dinT_ps = m2[:, 128:384]
                            for f in range(2):
                                for k in range(2):
                                    nc.tensor.matmul(
                                        dinT_ps[:, f * P:(f + 1) * P],
                                        lhsT=wfc1[k][:, f * P:(f + 1) * P],
                                        rhs=ctxT[:, k * P:(k + 1) * P],
                                        start=(k == 0), stop=(k == 1))
                            dinT = work.tile([P, 256], dt.bfloat16, tag="dinT",
                                             name="dinT")
                            nc.vector.tensor_copy(dinT[:], dinT_ps)

                            hh = hh12.tile([P, 512], dt.float32, tag="hh",
                                           name="hh")
                            h1_ps = hh[:, 0:256]
                            for f in range(2):
                                for k in range(2):
                                    nc.tensor.matmul(
                                        h1_ps[:, f * P:(f + 1) * P],
                                        lhsT=p1w1[k][:, f * P:(f + 1) * P],
                                        rhs=dinT[:, k * P:(k + 1) * P],
                                        start=(k == 0), stop=(k == 1))
                            h1T = work.tile([P, DM], dt.bfloat16, tag="h1T",
                                            name="h1T")
                            if flags["p1b1"]:
                                for f in range(2):
                                    nc.scalar.activation(
                                        h1T[:, f * P:(f + 1) * P],
                                        h1_ps[:, f * P:(f + 1) * P], AF.Tanh,
                                        bias=osb["p1b1"][:, f:f + 1])
                            else:
                                nc.scalar.activation(h1T[:], h1_ps[:], AF.Tanh)

                            nc.tensor.matmul(du_ps, lhsT=h1T[:, 0:P],
                                             rhs=p1w2[0][:], start=False,
                                             stop=False, skip_group_check=True)
                            nc.tensor.matmul(du_ps, lhsT=h1T[:, P:256],
                                             rhs=p1w2[1][:], start=False,
                                             stop=True, skip_group_check=True)
                            if flags["p1b2"]:
                                nc.vector.tensor_add(du_ps, du_ps,
                                                     osb["p1b2b"][:])
                            u_sl = u2_st[:, s * DM:(s + 1) * DM]
                            st6a = work.tile([P, 6], dt.float32, tag="st6a",
                                             name="st6a")
                            nc.vector.bn_stats(st6a[:], du_ps)
                            nc.scalar.activation(u_sl, du_ps, AF.Copy)
                            nc.vector.bn_aggr(mv_st[:, 4 * s:4 * s + 2],
                                              st6a[:])

                            h2_ps = hh[:, 256:512]
                            for f in range(2):
                                nc.tensor.matmul(h2_ps[:, f * P:(f + 1) * P],
                                                 lhsT=p2w1[:, f * P:(f + 1) * P],
                                                 rhs=emT)
                            h2T = work.tile([P, DM], dt.bfloat16, tag="h2T",
                                            name="h2T")
                            if flags["p2b1"]:
                                for f in range(2):
                                    nc.scalar.activation(
                                        h2T[:, f * P:(f + 1) * P],
                                        h2_ps[:, f * P:(f + 1) * P], AF.Tanh,
                                        bias=osb["p2b1"][:, f:f + 1])
                            else:
                                nc.scalar.activation(h2T[:], h2_ps[:], AF.Tanh)

                            st_ps = ds[:, 256:512]
                            nc.tensor.matmul(st_ps, lhsT=h2T[:, 0:P],
                                             rhs=p2w2[0][:], start=True,
                                             stop=False)
                            nc.tensor.matmul(st_ps, lhsT=h2T[:, P:256],
                                             rhs=p2w2[1][:], start=False,
                                             stop=True)
                            if flags["p2b2"]:
                                nc.vector.tensor_add(st_ps, st_ps,
                                                     osb["p2b2b"][:])
                            st_sl = st2_st[:, s * DM:(s + 1) * DM]
                            st6b = work.tile([P, 6], dt.float32, tag="st6b",
                                             name="st6b")
                            nc.vector.bn_stats(st6b[:], st_ps)
                            nc.scalar.activation(st_sl, st_ps, AF.Copy)
                            nc.vector.bn_aggr(mv_st[:, 4 * s + 2:4 * s + 4],
                                              st6b[:])

                        # defer this supertile's tail by one supertile so the
                        # serial gpsimd dd/dw/wsq chain never head-of-line
                        # blocks the vector queue at tensor_reduce
                        pend.append((gss, mv_st, u2_st, st2_st))
                        if len(pend) > 1:
                            emit_tail(*pend.pop(0))


                    while pend:
                        emit_tail(*pend.pop(0))

            # ---- final divide + store
            r3 = res[:].rearrange("p (t k) -> p t k", k=2)
            rn = work.tile([GRP, n_sub], dt.float32, tag="rn", name="rn")
            rn3 = rn[:].rearrange("p (t o) -> p t o", o=1)
            nc.vector.reciprocal(rn3[:], r3[:, :, 1:2])
            orow = work.tile([GRP, n_sub], dt.float32, tag="orow", name="orow")
            orow3 = orow[:].rearrange("p (t o) -> p t o", o=1)
            nc.vector.tensor_tensor(out=orow3[:], in0=r3[:, :, 0:1], in1=rn3[:],
                                    op=OP.mult)
            nc.sync.dma_start(outp[:, :], orow[:])

    nc.finalize()
    return nc


# ----------------------------------------------------------------- entry
_NC_CACHE = {}


def kernel(**inputs):
    _install_ntff_hook()
    from concourse.bass_utils import run_bass_kernel_spmd

    n_sub = int(os.environ.get("KBENCH_NSUB", NSUB_FULL))
    consts = _prep_consts(inputs)
    flags = consts.pop("_flags")
    bcls = consts.pop("_bcls")

    stage = int(os.environ.get("KBENCH_STAGE", "8"))
    key = (n_sub, stage, tuple(sorted(flags.items())))
    if key not in _NC_CACHE:
        _NC_CACHE[key] = build_nc(flags, bcls, n_sub, stage)
    nc = _NC_CACHE[key]

    x = np.asarray(inputs["x"]).astype(np.int32)
    in_maps = []
    for c in range(NCORES):
        xc = x[c * BC:(c + 1) * BC].reshape(-1)          # [16384]
        idxc = np.ascontiguousarray(
            xc[:n_sub * P].reshape(n_sub, P).T)          # [128, n_sub]
        m = {"idxc": idxc, "npmc": 0.5 * (idxc != 0).astype(np.float32)}
        m.update(consts)
        in_maps.append(m)

    trace = bool(int(os.environ.get("KBENCH_TRACE", "0")))
    res = run_bass_kernel_spmd(nc, in_maps, core_ids=list(range(NCORES)),
                               trace=trace)
    kernel._last_results = res

    out = np.zeros((B, 1), np.float32)
    for c in range(NCORES):
        oc = res.results[c]["outp"]                      # [8, n_sub]
        out[c * BC:c * BC + n_sub * GRP, 0] = oc.T.reshape(-1)
    return out
